# revision 32
# baseline (speedup 1.0000x reference)
"""Trainium2 Bass kernel for nn_EndpointVectorField (GVP message-passing GNN).

Strategy (8 NeuronCores, SPMD):
  - Edges sharded by pair: core c owns upper edges [c*10k,(c+1)*10k) and their
    reverse mates, sorted by dst and packed into 128-edge tiles that never split
    a dst group (enables race-free scatter via selection-matrix matmul).
  - All activations feature-on-partition [128, cols]; node tables in DRAM are
    gathered per edge via indirect DMA and transposed on the PE.
  - Per-node aggregation: per-core partial tables scattered to DRAM, AllReduce
    across cores, node update replicated on every core.
  - Vector channel (GVP) path packed as block-diagonal matmuls over the 3
    coords with Wh and Wh@Wu fused side by side.
"""
import sys
import numpy as np

if '/opt/trn_rl_repo' not in sys.path:
    sys.path.insert(0, '/opt/trn_rl_repo')

# ---- problem constants (hardcoded per contract) ----
N = 10000
EH = 80000
E = 2 * EH
HS = 128
RBF = 16
RBF_DMAX = 20.0
NC8 = 8
EC = E // NC8          # 20000 edges/core
NT = 164               # 128-edge tiles per core
EP = NT * 128          # 20992 padded edges
NST = NT // 4          # 41 super-tiles of 512
NROWS = 10240          # padded node rows
NNT = NROWS // 512     # 20 node super-tiles
TRASH = N
LN_EPS = 1e-5
F32 = np.float32

_CACHE = {}


def f32(x):
    return np.asarray(x, dtype=F32)


# ================= weight packing (validated in np sim) =================

def pack_gvp(p, vi_has_extra, dh, vo):
    Wh = f32(p['Wh']); Wu = f32(p['Wu'])
    WhWu = f32(Wh @ Wu)
    v_off = 1 if vi_has_extra else 0
    Wvh = np.zeros((48, 3 * dh), F32)
    Wvu = np.zeros((48, 3 * vo), F32)
    for c in range(3):
        for v in range(16):
            Wvh[c * 16 + v, c * dh:(c + 1) * dh] = Wh[v_off + v]
            Wvu[c * 16 + v, c * vo:(c + 1) * vo] = WhWu[v_off + v]
    out = {'Wvh': Wvh, 'Wvu': Wvu}
    if vi_has_extra:
        Wvh_x = np.zeros((3, 3 * dh), F32)
        Wvu_x = np.zeros((3, 3 * vo), F32)
        for c in range(3):
            Wvh_x[c, c * dh:(c + 1) * dh] = Wh[0]
            Wvu_x[c, c * vo:(c + 1) * vo] = WhWu[0]
        out['Wvh_x'] = Wvh_x
        out['Wvu_x'] = Wvu_x
    W = f32(p['lin']['W'])
    fi = W.shape[0] - dh
    out['Wlin_s'] = W[:fi]
    out['Wlin_sh'] = W[fi:]
    out['blin'] = f32(p['lin']['b'])
    Wg = f32(p['gate']['W']); bg = f32(p['gate']['b'])
    Wg_rep = np.zeros((Wg.shape[0], 3 * vo), F32)
    bg_rep = np.zeros((3 * vo,), F32)
    for c in range(3):
        Wg_rep[:, c * vo:(c + 1) * vo] = Wg
        bg_rep[c * vo:(c + 1) * vo] = bg
    out['Wg'] = Wg_rep
    out['bg'] = bg_rep
    out['dh'] = dh; out['vo'] = vo
    return out


def pack_weights(params):
    pk = {}
    for nm, src in [('scal_l1', params['scal_emb']['l1']),
                    ('scal_l2', params['scal_emb']['l2']),
                    ('edge_l1', params['edge_emb']['l1']),
                    ('edge_l2', params['edge_emb']['l2']),
                    ('eu_l2', params['edge_upd']['l2']),
                    ('nh_l1', params['node_head']['l1']),
                    ('nh_l2', params['node_head']['l2']),
                    ('eh_l1', params['edge_head']['l1']),
                    ('eh_l2', params['edge_head']['l2'])]:
        pk[nm] = (f32(src['W']), f32(src['b']))
    for nm, src in [('scal_ln', params['scal_emb']['ln']),
                    ('edge_ln', params['edge_emb']['ln']),
                    ('eu_ln', params['edge_upd']['ln'])]:
        pk[nm] = (f32(src['g']), f32(src['b']))
    pk['convs'] = []
    for ci in range(4):
        cv = params['convs'][ci]
        msg1 = pack_gvp(cv['msg'][0], True, 17, 16)
        W = msg1['Wlin_s']
        msg1['Ws'] = W[:HS].copy()
        msg1['Wd'] = W[HS:HS + RBF].copy()
        msg1['Wef'] = W[HS + RBF:].copy()
        msg = [msg1, pack_gvp(cv['msg'][1], False, 16, 16),
               pack_gvp(cv['msg'][2], False, 16, 16)]
        upd = [pack_gvp(cv['upd'][k], False, 16, 16) for k in range(3)]
        pk['convs'].append({
            'msg': msg, 'upd': upd,
            'ln_msg': (f32(cv['ln_msg']['g']), f32(cv['ln_msg']['b'])),
            'ln_upd': (f32(cv['ln_upd']['g']), f32(cv['ln_upd']['b']))})
    pk['pos'] = [pack_gvp(params['pos_upd'][0], False, 16, 16),
                 pack_gvp(params['pos_upd'][1], False, 16, 16),
                 pack_gvp(params['pos_upd'][2], False, 16, 1)]
    eu = params['edge_upd']
    W1 = f32(eu['l1']['W'])
    pk['eu_A'] = W1[:HS].copy()
    pk['eu_B'] = W1[HS:2 * HS].copy()
    pk['eu_C'] = W1[2 * HS:].copy()
    pk['eu_b1'] = f32(eu['l1']['b'])
    return pk


class Blob:
    """Packs 2-D f32 matrices into one [128, cols] SBUF-resident blob."""

    def __init__(self):
        self.cols = 0
        self.entries = {}   # name -> (row0, K, col0, M)
        self.arrays = {}

    def add(self, name, arr, row0=0):
        arr = f32(arr)
        if arr.ndim == 1:
            arr = arr[:, None]
        K, M = arr.shape
        assert row0 + K <= 128
        self.entries[name] = (row0, K, self.cols, M)
        self.arrays[name] = arr
        self.cols += M
        return name

    def finalize(self):
        buf = np.zeros((128, self.cols), F32)
        for name, (r0, K, c0, M) in self.entries.items():
            buf[r0:r0 + K, c0:c0 + M] = self.arrays[name]
        return buf


def build_blob(pk):
    B = Blob()
    B.add('ones128', np.ones((128, 1), F32))
    B.add('ones1', np.ones((1, 128), F32))
    ssel17 = np.zeros((51, 17), F32)
    for c in range(3):
        ssel17[c * 17:(c + 1) * 17] = np.eye(17, dtype=F32)
    B.add('ssel17', ssel17)
    ssel16 = np.zeros((48, 16), F32)
    for c in range(3):
        ssel16[c * 16:(c + 1) * 16] = np.eye(16, dtype=F32)
    B.add('ssel16', ssel16)
    mu = np.linspace(0.0, RBF_DMAX, RBF, dtype=F32)
    B.add('mu16', np.broadcast_to(mu[None, :], (128, RBF)).copy())
    B.add('eps8', np.full((128, 1), 1e-8, F32))
    B.add('epsln', np.full((128, 1), LN_EPS, F32))

    def add_gvp(pref, g):
        B.add(pref + 'Wvh', g['Wvh'])
        B.add(pref + 'Wvu', g['Wvu'])
        if 'Wvh_x' in g:
            B.add(pref + 'Wvh_x', g['Wvh_x'], row0=32)
            B.add(pref + 'Wvu_x', g['Wvu_x'], row0=32)
        if 'Ws' not in g and g['Wlin_s'].shape[0] in (128, 16):
            B.add(pref + 'Wls', g['Wlin_s'])
        B.add(pref + 'Wlsh', g['Wlin_sh'])
        B.add(pref + 'blin', g['blin'])
        B.add(pref + 'Wg', g['Wg'])
        B.add(pref + 'bg', g['bg'])

    for ci in range(4):
        cv = pk['convs'][ci]
        m1 = cv['msg'][0]
        B.add(f'c{ci}z1w', m1['Ws'])
        B.add(f'c{ci}m1Wd', m1['Wd'])
        B.add(f'c{ci}m1Wef', m1['Wef'])
        add_gvp(f'c{ci}m1', m1)
        add_gvp(f'c{ci}m2', cv['msg'][1])
        add_gvp(f'c{ci}m3', cv['msg'][2])
        for k in range(3):
            add_gvp(f'c{ci}u{k}', cv['upd'][k])
        B.add(f'c{ci}lnmg', cv['ln_msg'][0])
        B.add(f'c{ci}lnmb', cv['ln_msg'][1])
        B.add(f'c{ci}lnug', cv['ln_upd'][0])
        B.add(f'c{ci}lnub', cv['ln_upd'][1])
    for k in range(3):
        add_gvp(f'p{k}', pk['pos'][k])
    B.add('euA', pk['eu_A'])
    B.add('euB', pk['eu_B'])
    B.add('euC', pk['eu_C'])
    B.add('eub1', pk['eu_b1'])
    for nm in ['eu_l2', 'nh_l1', 'nh_l2', 'eh_l1', 'eh_l2',
               'scal_l1', 'scal_l2', 'edge_l1', 'edge_l2']:
        B.add(nm + 'W', pk[nm][0])
        B.add(nm + 'b', pk[nm][1])
    for nm in ['scal_ln', 'edge_ln', 'eu_ln']:
        B.add(nm + 'g', pk[nm][0])
        B.add(nm + 'b', pk[nm][1])
    return B


# ================= host prep =================

def prep(inputs):
    src = np.asarray(inputs['src_idx']).astype(np.int64)
    dst = np.asarray(inputs['dst_idx']).astype(np.int64)
    e_t = f32(inputs['e_t'])
    a_t = f32(inputs['a_t']); c_t = f32(inputs['c_t'])
    x_t = f32(inputs['x_t']); t = f32(inputs['t'])
    nbi = np.asarray(inputs['node_batch_idx']).astype(np.int64)

    s0 = np.zeros((N, 17), F32)
    s0[:, :10] = a_t
    s0[:, 10] = t[nbi]
    s0[:, 11:] = c_t
    s0T = np.zeros((17, NROWS), F32)
    s0T[:, :N] = s0.T
    x0T = np.zeros((3, NROWS), F32)
    x0T[:, :N] = x_t.T

    mu = np.linspace(0.0, RBF_DMAX, RBF, dtype=F32)
    sigma = F32(RBF_DMAX / RBF)

    cores = []
    for c in range(NC8):
        PH = EC // 2
        gidx = np.concatenate([np.arange(c * PH, (c + 1) * PH),
                               EH + np.arange(c * PH, (c + 1) * PH)])
        sc = src[gidx]; dc = dst[gidx]
        order = np.argsort(dc, kind='stable')
        ds = dc[order]
        groups = []
        run = 0
        for i in range(1, len(ds) + 1):
            if i == len(ds) or ds[i] != ds[i - 1]:
                groups.append((run, i)); run = i
        tiles = []
        cur = []; cur_len = 0
        for (a, b) in groups:
            gl = b - a
            assert gl <= 128, f"in-degree {gl} > 128"
            if cur_len + gl > 128:
                tiles.append(np.concatenate(cur)); cur = []; cur_len = 0
            cur.append(order[a:b]); cur_len += gl
        if cur_len:
            tiles.append(np.concatenate(cur))
        assert len(tiles) <= NT, f"core {c}: {len(tiles)} tiles > {NT}"
        src_p = np.zeros(EP, np.int32)
        dst_p = np.full(EP, TRASH, np.int32)
        e0_p = np.zeros((EP, 5), F32)
        gid_p = np.full(EP, -1, np.int64)
        for ti, tl in enumerate(tiles):
            n = len(tl)
            src_p[ti * 128:ti * 128 + n] = sc[tl]
            dst_p[ti * 128:ti * 128 + n] = dc[tl]
            e0_p[ti * 128:ti * 128 + n] = e_t[gidx[tl]]
            gid_p[ti * 128:ti * 128 + n] = gidx[tl]
        pos_of = {}
        for p_, g_ in enumerate(gid_p):
            if g_ >= 0:
                pos_of[g_] = p_
        up = np.arange(c * PH, (c + 1) * PH)
        U = np.zeros(NROWS, np.int32)
        M = np.zeros(NROWS, np.int32)
        U[:PH] = [pos_of[g_] for g_ in up]
        M[:PH] = [pos_of[g_ + EH] for g_ in up]

        diff = (x_t[src_p] - np.where((dst_p < N)[:, None],
                                      x_t[np.minimum(dst_p, N - 1)], 0.0)).astype(F32)
        dij = (np.sqrt((diff * diff).sum(1) + F32(1e-8)) + F32(1e-8)).astype(F32)
        xdf = (diff / dij[:, None]).astype(F32)
        d0 = np.exp(-(((dij[:, None] - mu[None, :]) / sigma) ** 2)).astype(F32)

        aux0 = np.zeros((35, EP), F32)
        aux0[0:16] = d0.T
        aux0[32:35] = xdf.T

        cores.append({
            'aux0': aux0,
            'dstf': dst_p.astype(F32).reshape(NT, 128).T.copy(),
            'srci': src_p.reshape(NT, 128).T.copy(),
            'dsti': dst_p.reshape(NT, 128).T.copy(),
            'Ui': U.reshape(NROWS // 128, 128).T.copy(),
            'Mi': M.reshape(NROWS // 128, 128).T.copy(),
            'e0T': np.ascontiguousarray(e0_p.T),
        })
    return cores, s0T, x0T


# ================= device program =================

def build_program(blob_entries, wcols):
    import concourse.bass as bass
    import concourse.bacc as bacc
    import concourse.tile as tile
    from concourse import mybir
    from concourse.masks import make_identity

    AF = mybir.ActivationFunctionType
    ALU = mybir.AluOpType
    DT = mybir.dt

    nc = bacc.Bacc("TRN2", target_bir_lowering=False, debug=False,
                   num_devices=NC8, enable_asserts=False)

    # ---- I/O ----
    WB = nc.dram_tensor("WB", [128, wcols], DT.float32, kind="ExternalInput")
    s0T = nc.dram_tensor("s0T", [17, NROWS], DT.float32, kind="ExternalInput")
    aux0 = nc.dram_tensor("aux0", [35, EP], DT.float32, kind="ExternalInput")
    x0T_in = nc.dram_tensor("x0T", [3, NROWS], DT.float32, kind="ExternalInput")
    dstf_in = nc.dram_tensor("dstf", [128, NT], DT.float32, kind="ExternalInput")
    srci_in = nc.dram_tensor("srci", [128, NT], DT.int32, kind="ExternalInput")
    dsti_in = nc.dram_tensor("dsti", [128, NT], DT.int32, kind="ExternalInput")
    Ui_in = nc.dram_tensor("Ui", [128, NROWS // 128], DT.int32, kind="ExternalInput")
    Mi_in = nc.dram_tensor("Mi", [128, NROWS // 128], DT.int32, kind="ExternalInput")
    e0T_in = nc.dram_tensor("e0T", [5, EP], DT.float32, kind="ExternalInput")

    out_x = nc.dram_tensor("out_x", [NROWS, 3], DT.float32, kind="ExternalOutput")
    out_nh = nc.dram_tensor("out_nh", [NROWS, 16], DT.float32, kind="ExternalOutput")
    out_el = nc.dram_tensor("out_el", [NROWS, 5], DT.float32, kind="ExternalOutput")

    # ---- internal DRAM tables ----
    TAB_Z1 = nc.dram_tensor("TAB_Z1", [NROWS, 128], DT.float32, kind="Internal")
    TAB_V = nc.dram_tensor("TAB_V", [NROWS, 48], DT.float32, kind="Internal")
    TAB_ZA = nc.dram_tensor("TAB_ZA", [NROWS, 128], DT.float32, kind="Internal")
    TAB_ZB = nc.dram_tensor("TAB_ZB", [NROWS, 128], DT.float32, kind="Internal")
    TAB_X = nc.dram_tensor("TAB_X", [NROWS, 3], DT.float32, kind="Internal")
    TAB_S = nc.dram_tensor("TAB_S", [128, NROWS], DT.float32, kind="Internal")
    TAB_VF = nc.dram_tensor("TAB_VF", [48, NROWS], DT.float32, kind="Internal")
    TAB_EF = nc.dram_tensor("TAB_EF", [128, EP], DT.float32, kind="Internal")
    TAB_EFM = nc.dram_tensor("TAB_EFM", [EP, 128], DT.float32, kind="Internal")
    AUXD2 = nc.dram_tensor("AUXD2", [35, EP], DT.float32, kind="Internal")
    XF = nc.dram_tensor("XF", [3, NROWS], DT.float32, kind="Internal")
    AGG_IN = [nc.dram_tensor(f"AGG_IN{ci}", [NROWS, 176], DT.float32, kind="Internal")
              for ci in range(4)]
    AGG_OUT = [nc.dram_tensor(f"AGG_OUT{ci}", [NROWS, 176], DT.float32,
                              kind="Internal", addr_space="Shared")
               for ci in range(4)]

    with tile.TileContext(nc) as tc:
        from contextlib import ExitStack
        ctx = ExitStack()
        with ctx:
            persist = ctx.enter_context(tc.tile_pool(name="persist", bufs=1))
            sb = ctx.enter_context(tc.tile_pool(name="sb", bufs=1))
            sb2 = ctx.enter_context(tc.tile_pool(name="sb2", bufs=2))
            gat = ctx.enter_context(tc.tile_pool(name="gat", bufs=8))
            ps = ctx.enter_context(tc.tile_pool(name="ps", bufs=1, space="PSUM"))

            wb = persist.tile([128, wcols], DT.float32)
            nc.sync.dma_start(out=wb[:], in_=WB[:, :])

            def W(name):
                r0, K, c0, M = blob_entries[name]
                return wb[r0:r0 + K, c0:c0 + M]

            dstf = persist.tile([128, NT], DT.float32)
            srci = persist.tile([128, NT], DT.int32)
            dsti = persist.tile([128, NT], DT.int32)
            Ui = persist.tile([128, NROWS // 128], DT.int32)
            Mi = persist.tile([128, NROWS // 128], DT.int32)
            for t_, i_ in [(dstf, dstf_in), (srci, srci_in), (dsti, dsti_in),
                           (Ui, Ui_in), (Mi, Mi_in)]:
                nc.sync.dma_start(out=t_[:], in_=i_[:, :])

            ident = persist.tile([128, 128], DT.float32)
            make_identity(nc, ident[:])
            zero_sb = persist.tile([128, 176], DT.float32)
            nc.vector.memset(zero_sb[:], 0.0)

            MM = nc.tensor.matmul

            # ---------- helpers ----------
            def ln_cols(pre_sb, gname, bname, out_t, n=512):
                """LayerNorm over partitions for [128, n] tile -> out_t."""
                stp = ps.tile([33, 512], DT.float32, tag="stats")
                stats = stp[0:1, :]
                sqs = stp[32:33, :]
                sq_sb = sb.tile([128, 512], DT.float32, tag="lnw")
                MM(out=stats[:, :n], lhsT=W('ones128'), rhs=pre_sb, start=True, stop=True)
                nc.scalar.activation(out=sq_sb[:, :n], in_=pre_sb, func=AF.Square)
                MM(out=sqs[:, :n], lhsT=W('ones128'), rhs=sq_sb[:, :n], start=True, stop=True)
                st_ = sb.tile([1, 2048], DT.float32, tag="lnst")
                m_sb = st_[0:1, 0:n]
                v_sb = st_[0:1, 512:512 + n]
                m2 = st_[0:1, 1024:1024 + n]
                r_sb = st_[0:1, 1536:1536 + n]
                nc.scalar.activation(out=m_sb, in_=stats[:, :n], func=AF.Copy,
                                     scale=1.0 / 128.0)
                nc.scalar.activation(out=v_sb, in_=sqs[:, :n], func=AF.Copy,
                                     scale=1.0 / 128.0)
                nc.vector.tensor_mul(out=m2, in0=m_sb, in1=m_sb)
                nc.vector.tensor_sub(out=v_sb, in0=v_sb, in1=m2)
                nc.scalar.activation(out=v_sb, in_=v_sb, func=AF.Sqrt,
                                     bias=W('epsln')[0:1, :])
                nc.vector.reciprocal(out=r_sb, in_=v_sb)
                mb = ps.tile([128, 512], DT.float32, tag="scat")
                rb = ps.tile([128, 512], DT.float32, tag="red")
                MM(out=mb[:, :n], lhsT=W('ones1'), rhs=m_sb, start=True, stop=True)
                MM(out=rb[:, :n], lhsT=W('ones1'), rhs=r_sb, start=True, stop=True)
                cs = sb.tile([128, 512], DT.float32, tag="lnw")
                nc.vector.tensor_sub(out=cs[:, :n], in0=pre_sb, in1=mb[:, :n])
                nc.vector.tensor_mul(out=cs[:, :n], in0=cs[:, :n], in1=rb[:, :n])
                nc.scalar.activation(out=out_t, in_=cs[:, :n], func=AF.Identity,
                                     scale=W(gname)[:, :], bias=W(bname)[:, :])

            def gvp(pref, dh, vo, lin_ins, mv_sb_ap, xdf_ap, out_ms, sigmoid_gate=True):
                """One GVP. lin_ins: list of (lhsT_name_or_ap, rhs_ap, K) matmul
                contributions plus optional ('T', src_tile) transpose contribs.
                mv_sb_ap: [48, 512] SBUF. Returns (gate_or_sig_sb, vu_psum)."""
                vh = ps.tile([51, 512], DT.float32, tag="vh")
                vu = ps.tile([48, 512], DT.float32, tag="vu")
                MM(out=vh[:3 * dh, :], lhsT=W(pref + 'Wvh'), rhs=mv_sb_ap,
                   start=True, stop=(xdf_ap is None))
                MM(out=vu[:3 * vo, :], lhsT=W(pref + 'Wvu'), rhs=mv_sb_ap,
                   start=True, stop=(xdf_ap is None))
                if xdf_ap is not None:
                    MM(out=vh[:3 * dh, :], lhsT=W(pref + 'Wvh_x'), rhs=xdf_ap,
                       start=False, stop=True)
                    MM(out=vu[:3 * vo, :], lhsT=W(pref + 'Wvu_x'), rhs=xdf_ap,
                       start=False, stop=True)
                sq = sb2.tile([51, 512], DT.float32, tag="sq")
                nc.scalar.activation(out=sq[:3 * dh, :], in_=vh[:3 * dh, :], func=AF.Square)
                ssq = ps.tile([17, 512], DT.float32, tag="gvaux")
                sselw = 'ssel17' if dh == 17 else 'ssel16'
                MM(out=ssq[:dh, :], lhsT=W(sselw), rhs=sq[:3 * dh, :], start=True, stop=True)
                sh = sb2.tile([17, 512], DT.float32, tag="sh")
                nc.scalar.activation(out=sh[:dh, :], in_=ssq[:dh, :], func=AF.Sqrt,
                                     bias=W('eps8')[0:dh, :])
                lin = ps.tile([128, 512], DT.float32, tag="lin")
                first = True
                for item in lin_ins:
                    if item[0] == 'T':
                        assert not first, "transposes must accumulate after a start"
                        for j, zt in enumerate(item[1]):
                            MM(out=lin[:, j * 128:(j + 1) * 128], lhsT=zt,
                               rhs=ident[:], is_transpose=True,
                               start=False, stop=False, skip_group_check=True)
                    else:
                        lname, rhs_ap = item
                        MM(out=lin[:], lhsT=W(lname), rhs=rhs_ap,
                           start=first, stop=False, skip_group_check=True)
                        first = False
                MM(out=lin[:], lhsT=W(pref + 'Wlsh'), rhs=sh[:dh, :],
                   start=False, stop=True, skip_group_check=True)
                nc.scalar.activation(out=out_ms, in_=lin[:], func=AF.Silu,
                                     bias=W(pref + 'blin')[:, :])
                gate = ps.tile([48, 512], DT.float32, tag="gvaux")
                MM(out=gate[:3 * vo, :], lhsT=W(pref + 'Wg'), rhs=out_ms,
                   start=True, stop=True)
                sig = sb2.tile([48, 512], DT.float32, tag="sig")
                nc.scalar.activation(out=sig[:3 * vo, :], in_=gate[:3 * vo, :],
                                     func=AF.Sigmoid if sigmoid_gate else AF.Identity,
                                     bias=W(pref + 'bg')[:3 * vo, :])
                return sig, vu

            def gvp_chain2(pref2, pref3, ms_in, mv_in, out_ms, out_mv):
                """GVP2 then GVP3 (dh=vo=16), edge or node side."""
                ms2 = sb2.tile([128, 512], DT.float32, tag="ms2")
                sig2, vu2 = gvp(pref2, 16, 16, [(pref2 + 'Wls', ms_in)], mv_in, None, ms2[:])
                mv2 = sb2.tile([48, 512], DT.float32, tag="mv2")
                nc.vector.tensor_mul(out=mv2[:], in0=sig2[:], in1=vu2[:])
                sig3, vu3 = gvp(pref3, 16, 16, [(pref3 + 'Wls', ms2[:])], mv2[:], None, out_ms)
                nc.vector.tensor_mul(out=out_mv, in0=sig3[:], in1=vu3[:])

            def write_nodemaj(src_sb, tab, nt, width):
                """[width<=128, 512] feature-major tile -> node-major DRAM rows."""
                for j in range(4):
                    tp = ps.tile([128, 128], DT.float32, tag="wtp")
                    MM(out=tp[:, :width], lhsT=src_sb[:, j * 128:(j + 1) * 128],
                       rhs=ident[:width, :width], is_transpose=True, start=True, stop=True)
                    ev = sb.tile([128, 128], DT.float32, tag="wev")
                    nc.vector.tensor_copy(out=ev[:, :width], in_=tp[:, :width])
                    nc.sync.dma_start(
                        out=tab[nt * 512 + j * 128: nt * 512 + (j + 1) * 128, :],
                        in_=ev[:, :width])

            # ---------- init node phase: scal_emb ----------
            for nt in range(NNT):
                sl = slice(nt * 512, (nt + 1) * 512)
                s0_sb = sb.tile([17, 512], DT.float32, tag="s0")
                nc.sync.dma_start(out=s0_sb[:], in_=s0T[:, sl])
                p1 = ps.tile([128, 512], DT.float32, tag="lin")
                MM(out=p1[:], lhsT=W('scal_l1W'), rhs=s0_sb[:], start=True, stop=True)
                h1 = sb.tile([128, 512], DT.float32, tag="h1")
                nc.scalar.activation(out=h1[:], in_=p1[:], func=AF.Silu,
                                     bias=W('scal_l1b')[:, :])
                p2 = ps.tile([128, 512], DT.float32, tag="lin")
                MM(out=p2[:], lhsT=W('scal_l2W'), rhs=h1[:], start=True, stop=True)
                h2 = sb.tile([128, 512], DT.float32, tag="h2")
                nc.scalar.activation(out=h2[:], in_=p2[:], func=AF.Silu,
                                     bias=W('scal_l2b')[:, :])
                s_sb = sb.tile([128, 512], DT.float32, tag="sout")
                ln_cols(h2[:], 'scal_lng', 'scal_lnb', s_sb[:])
                nc.sync.dma_start(out=TAB_S[:, sl], in_=s_sb[:])
                z1 = ps.tile([128, 512], DT.float32, tag="lin")
                MM(out=z1[:], lhsT=W('c0z1w'), rhs=s_sb[:], start=True, stop=True)
                z1s = sb.tile([128, 512], DT.float32, tag="z1s")
                nc.vector.tensor_copy(out=z1s[:], in_=z1[:])
                write_nodemaj(z1s[:], TAB_Z1, nt, 128)
                # zero V tables + TAB_X / XF init
                xb = sb.tile([3, 512], DT.float32, tag="xb")
                nc.sync.dma_start(out=xb[:], in_=x0T_in[:, sl])
                nc.sync.dma_start(out=XF[:, sl], in_=xb[:])
                for j in range(4):
                    r0 = nt * 512 + j * 128
                    nc.sync.dma_start(out=TAB_V[r0:r0 + 128, :], in_=zero_sb[:, :48])
                    xt = ps.tile([128, 128], DT.float32, tag="wtp")
                    MM(out=xt[:, :3], lhsT=xb[:, j * 128:(j + 1) * 128],
                       rhs=ident[0:3, 0:3], is_transpose=True, start=True, stop=True)
                    xe = sb.tile([128, 128], DT.float32, tag="wev")
                    nc.vector.tensor_copy(out=xe[:, :3], in_=xt[:, :3])
                    nc.sync.dma_start(out=TAB_X[r0:r0 + 128, :], in_=xe[:, :3])
                vz = sb2.tile([48, 512], DT.float32, tag="mv3")
                nc.vector.memset(vz[:], 0.0)
                nc.sync.dma_start(out=TAB_VF[:, sl], in_=vz[:])

            # ---------- init edge phase: edge_emb ----------
            for st in range(NST):
                sl = slice(st * 512, (st + 1) * 512)
                e0_sb = sb.tile([5, 512], DT.float32, tag="e0")
                nc.sync.dma_start(out=e0_sb[:], in_=e0T_in[:, sl])
                p1 = ps.tile([128, 512], DT.float32, tag="lin")
                MM(out=p1[:], lhsT=W('edge_l1W'), rhs=e0_sb[:], start=True, stop=True)
                h1 = sb.tile([128, 512], DT.float32, tag="h1")
                nc.scalar.activation(out=h1[:], in_=p1[:], func=AF.Silu,
                                     bias=W('edge_l1b')[:, :])
                p2 = ps.tile([128, 512], DT.float32, tag="lin")
                MM(out=p2[:], lhsT=W('edge_l2W'), rhs=h1[:], start=True, stop=True)
                h2 = sb.tile([128, 512], DT.float32, tag="h2")
                nc.scalar.activation(out=h2[:], in_=p2[:], func=AF.Silu,
                                     bias=W('edge_l2b')[:, :])
                ef_sb = sb.tile([128, 512], DT.float32, tag="efout")
                ln_cols(h2[:], 'edge_lng', 'edge_lnb', ef_sb[:])
                nc.sync.dma_start(out=TAB_EF[:, sl], in_=ef_sb[:])

            # ---------- conv loop ----------
            for ci in range(4):
                agg_in = AGG_IN[ci]
                agg_out = AGG_OUT[ci]
                # zero the partial table
                for r in range(NROWS // 128):
                    nc.sync.dma_start(out=agg_in[r * 128:(r + 1) * 128, :],
                                      in_=zero_sb[:, :])

                # ---- edge message phase ----
                for st in range(NST):
                    sl = slice(st * 512, (st + 1) * 512)
                    zts = []
                    vg_ts = []
                    for j in range(4):
                        tcol = st * 4 + j
                        zg = gat.tile([128, 128], DT.float32, tag="zg")
                        nc.gpsimd.indirect_dma_start(
                            out=zg[:], out_offset=None, in_=TAB_Z1[:, :],
                            in_offset=bass.IndirectOffsetOnAxis(
                                ap=srci[:, tcol:tcol + 1], axis=0))
                        zts.append(zg)
                        vg = gat.tile([128, 48], DT.float32, tag="vg")
                        nc.gpsimd.indirect_dma_start(
                            out=vg[:], out_offset=None, in_=TAB_V[:, :],
                            in_offset=bass.IndirectOffsetOnAxis(
                                ap=srci[:, tcol:tcol + 1], axis=0))
                        vg_ts.append(vg)
                    ef_sb = sb2.tile([128, 512], DT.float32, tag="ef")
                    nc.sync.dma_start(out=ef_sb[:], in_=TAB_EF[:, sl])
                    dxf = sb2.tile([35, 512], DT.float32, tag="dxf")
                    nc.sync.dma_start(out=dxf[:],
                                      in_=(aux0 if ci < 2 else AUXD2)[:, sl])
                    # V transpose -> mv0 [48, 512]
                    vtp = ps.tile([48, 512], DT.float32, tag="gvaux")
                    for j in range(4):
                        MM(out=vtp[:, j * 128:(j + 1) * 128], lhsT=vg_ts[j][:],
                           rhs=ident[:], is_transpose=True, start=True, stop=True)
                    mv0 = sb2.tile([48, 512], DT.float32, tag="mv0")
                    nc.vector.tensor_copy(out=mv0[:], in_=vtp[:])

                    ms1 = sb2.tile([128, 512], DT.float32, tag="ms1")
                    sig1, vu1 = gvp(
                        f'c{ci}m1', 17, 16,
                        [(f'c{ci}m1Wef', ef_sb[:]),
                         (f'c{ci}m1Wd', dxf[0:16, :]),
                         ('T', [z[:] for z in zts])],
                        mv0[:], dxf[32:35, :], ms1[:])
                    mv1 = sb2.tile([48, 512], DT.float32, tag="mv1")
                    nc.vector.tensor_mul(out=mv1[:], in0=sig1[:], in1=vu1[:])
                    ms3 = sb2.tile([128, 512], DT.float32, tag="ms3")
                    mv3 = sb2.tile([48, 512], DT.float32, tag="mv3")
                    gvp_chain2(f'c{ci}m2', f'c{ci}m3', ms1[:], mv1[:], ms3[:], mv3[:])

                    # ---- scatter ----
                    for j in range(4):
                        tcol = st * 4 + j
                        sp = ps.tile([128, 512], DT.float32, tag="scat")
                        # dstT
                        MM(out=sp[:, 0:128],
                           lhsT=dstf[:, tcol:tcol + 1].to_broadcast([128, 128]),
                           rhs=ident[:], is_transpose=True, start=True, stop=True)
                        dstT = sb.tile([128, 128], DT.float32, tag="dstT")
                        nc.vector.tensor_copy(out=dstT[:], in_=sp[:, 0:128])
                        sel = sb.tile([128, 128], DT.float32, tag="sel")
                        nc.vector.tensor_tensor(
                            out=sel[:],
                            in0=dstf[:, tcol:tcol + 1].to_broadcast([128, 128]),
                            in1=dstT[:], op=ALU.is_equal)
                        # ms/mv transposes
                        MM(out=sp[:, 128:256], lhsT=ms3[:, j * 128:(j + 1) * 128],
                           rhs=ident[:], is_transpose=True, start=True, stop=True)
                        MM(out=sp[:, 256:304], lhsT=mv3[:, j * 128:(j + 1) * 128],
                           rhs=ident[:48, :48], is_transpose=True, start=True, stop=True)
                        ets = sb.tile([128, 176], DT.float32, tag="ets")
                        nc.vector.tensor_copy(out=ets[:], in_=sp[:, 128:304])
                        red = ps.tile([128, 176], DT.float32, tag="red")
                        MM(out=red[:, 0:128], lhsT=sel[:], rhs=ets[:, 0:128],
                           start=True, stop=True)
                        MM(out=red[:, 128:176], lhsT=sel[:], rhs=ets[:, 128:176],
                           start=True, stop=True)
                        redsb = sb.tile([128, 176], DT.float32, tag="redsb")
                        nc.scalar.activation(out=redsb[:], in_=red[:], func=AF.Copy,
                                             scale=0.01)
                        nc.gpsimd.indirect_dma_start(
                            out=agg_in[:, :],
                            out_offset=bass.IndirectOffsetOnAxis(
                                ap=dsti[:, tcol:tcol + 1], axis=0),
                            in_=redsb[:], in_offset=None)

                # ---- AllReduce ----
                nc.gpsimd.collective_compute(
                    "AllReduce", ALU.add,
                    replica_groups=[list(range(NC8))],
                    ins=[agg_in[:, :]], outs=[agg_out[:, :]])

                # ---- node phase ----
                for nt in range(NNT):
                    sl = slice(nt * 512, (nt + 1) * 512)
                    ams = ps.tile([128, 512], DT.float32, tag="scat")
                    amv = ps.tile([48, 512], DT.float32, tag="red")
                    for j in range(4):
                        r0 = nt * 512 + j * 128
                        ag = gat.tile([128, 176], DT.float32, tag="ag")
                        nc.sync.dma_start(out=ag[:], in_=agg_out[r0:r0 + 128, :])
                        MM(out=ams[:, j * 128:(j + 1) * 128], lhsT=ag[:, 0:128],
                           rhs=ident[:], is_transpose=True, start=True, stop=True)
                        MM(out=amv[:, j * 128:(j + 1) * 128], lhsT=ag[:, 128:176],
                           rhs=ident[:], is_transpose=True, start=True, stop=True)
                    s_sb = sb.tile([128, 512], DT.float32, tag="snode")
                    nc.sync.dma_start(out=s_sb[:], in_=TAB_S[:, sl])
                    pre = sb.tile([128, 512], DT.float32, tag="pre")
                    nc.vector.tensor_add(out=pre[:], in0=s_sb[:], in1=ams[:])
                    vf = sb.tile([48, 512], DT.float32, tag="vf")
                    nc.sync.dma_start(out=vf[:], in_=TAB_VF[:, sl])
                    vmid = sb.tile([48, 512], DT.float32, tag="vmid")
                    nc.vector.tensor_add(out=vmid[:], in0=vf[:], in1=amv[:])
                    s_ln = sb.tile([128, 512], DT.float32, tag="sln")
                    ln_cols(pre[:], f'c{ci}lnmg', f'c{ci}lnmb', s_ln[:])
                    # upd GVPs
                    us1 = sb.tile([128, 512], DT.float32, tag="us1")
                    sigu1, vuu1 = gvp(f'c{ci}u0', 16, 16,
                                      [(f'c{ci}u0Wls', s_ln[:])], vmid[:], None, us1[:])
                    uv1 = sb.tile([48, 512], DT.float32, tag="uv1")
                    nc.vector.tensor_mul(out=uv1[:], in0=sigu1[:], in1=vuu1[:])
                    us3 = sb.tile([128, 512], DT.float32, tag="us3")
                    uv3 = sb.tile([48, 512], DT.float32, tag="uv3")
                    gvp_chain2(f'c{ci}u1', f'c{ci}u2', us1[:], uv1[:], us3[:], uv3[:])
                    pre2 = sb.tile([128, 512], DT.float32, tag="pre2")
                    nc.vector.tensor_add(out=pre2[:], in0=s_ln[:], in1=us3[:])
                    s_out = sb.tile([128, 512], DT.float32, tag="sfin")
                    ln_cols(pre2[:], f'c{ci}lnug', f'c{ci}lnub', s_out[:])
                    v_out = sb.tile([48, 512], DT.float32, tag="vfin")
                    nc.vector.tensor_add(out=v_out[:], in0=vmid[:], in1=uv3[:])

                    if ci < 3:
                        nc.sync.dma_start(out=TAB_S[:, sl], in_=s_out[:])
                        nc.sync.dma_start(out=TAB_VF[:, sl], in_=v_out[:])
                        z1 = ps.tile([128, 512], DT.float32, tag="lin")
                        MM(out=z1[:], lhsT=W(f'c{ci + 1}z1w'), rhs=s_out[:],
                           start=True, stop=True)
                        z1s = sb.tile([128, 512], DT.float32, tag="z1s")
                        nc.vector.tensor_copy(out=z1s[:], in_=z1[:])
                        write_nodemaj(z1s[:], TAB_Z1, nt, 128)
                        write_nodemaj(v_out[:], TAB_V, nt, 48)

                    if ci in (1, 3):
                        # position update GVPs
                        ps1_ = sb.tile([128, 512], DT.float32, tag="ps1t")
                        sigp1, vup1 = gvp('p0', 16, 16, [('p0Wls', s_out[:])],
                                          v_out[:], None, ps1_[:])
                        pv1 = sb.tile([48, 512], DT.float32, tag="pv1")
                        nc.vector.tensor_mul(out=pv1[:], in0=sigp1[:], in1=vup1[:])
                        ps2_ = sb.tile([128, 512], DT.float32, tag="ps2t")
                        sigp2, vup2 = gvp('p1', 16, 16, [('p1Wls', ps1_[:])],
                                          pv1[:], None, ps2_[:])
                        pv2 = sb.tile([48, 512], DT.float32, tag="pv2")
                        nc.vector.tensor_mul(out=pv2[:], in0=sigp2[:], in1=vup2[:])
                        ps3_ = sb.tile([128, 512], DT.float32, tag="ps3t")
                        sigp3, vup3 = gvp('p2', 16, 1, [('p2Wls', ps2_[:])],
                                          pv2[:], None, ps3_[:], sigmoid_gate=False)
                        dx = sb.tile([3, 512], DT.float32, tag="dx")
                        nc.vector.tensor_mul(out=dx[:], in0=sigp3[:3, :], in1=vup3[:3, :])
                        xb = sb.tile([3, 512], DT.float32, tag="xb")
                        nc.sync.dma_start(out=xb[:], in_=XF[:, sl])
                        nc.vector.tensor_add(out=xb[:], in0=xb[:], in1=dx[:])
                        nc.sync.dma_start(out=XF[:, sl], in_=xb[:])
                        # za/zb tables for edge update
                        for wnm, tab in [('euA', TAB_ZA), ('euB', TAB_ZB)]:
                            zp = ps.tile([128, 512], DT.float32, tag="lin")
                            MM(out=zp[:], lhsT=W(wnm), rhs=s_out[:], start=True, stop=True)
                            zs = sb.tile([128, 512], DT.float32, tag="z1s")
                            nc.vector.tensor_copy(out=zs[:], in_=zp[:])
                            write_nodemaj(zs[:], tab, nt, 128)
                        if ci == 1:
                            # refresh TAB_X for dist recompute
                            for j in range(4):
                                r0 = nt * 512 + j * 128
                                xt = ps.tile([128, 128], DT.float32, tag="wtp")
                                MM(out=xt[:, :3], lhsT=xb[:, j * 128:(j + 1) * 128],
                                   rhs=ident[0:3, 0:3], is_transpose=True,
                                   start=True, stop=True)
                                xe = sb.tile([128, 128], DT.float32, tag="wev")
                                nc.vector.tensor_copy(out=xe[:, :3], in_=xt[:, :3])
                                nc.sync.dma_start(out=TAB_X[r0:r0 + 128, :], in_=xe[:, :3])

                    if ci == 3:
                        # node head
                        hp = ps.tile([128, 512], DT.float32, tag="lin")
                        MM(out=hp[:], lhsT=W('nh_l1W'), rhs=s_out[:], start=True, stop=True)
                        hh = sb.tile([128, 512], DT.float32, tag="h1")
                        nc.scalar.activation(out=hh[:], in_=hp[:], func=AF.Silu,
                                             bias=W('nh_l1b')[:, :])
                        op = ps.tile([16, 512], DT.float32, tag="wtp")
                        MM(out=op[:], lhsT=W('nh_l2W'), rhs=hh[:], start=True, stop=True)
                        ob = sb.tile([16, 512], DT.float32, tag="nhsb")
                        nc.scalar.activation(out=ob[:], in_=op[:], func=AF.Identity,
                                             bias=W('nh_l2b')[:, :])
                        for j in range(4):
                            r0 = nt * 512 + j * 128
                            tp = ps.tile([128, 128], DT.float32, tag="wtp")
                            MM(out=tp[:, :16], lhsT=ob[:, j * 128:(j + 1) * 128],
                               rhs=ident[:16, :16], is_transpose=True, start=True, stop=True)
                            ev = sb.tile([128, 128], DT.float32, tag="wev")
                            nc.vector.tensor_copy(out=ev[:, :16], in_=tp[:, :16])
                            nc.sync.dma_start(out=out_nh[r0:r0 + 128, :], in_=ev[:, :16])
                            # x output
                            xt = ps.tile([128, 128], DT.float32, tag="wtp")
                            MM(out=xt[:, :3], lhsT=xb[:, j * 128:(j + 1) * 128],
                               rhs=ident[0:3, 0:3], is_transpose=True,
                               start=True, stop=True)
                            xe = sb.tile([128, 128], DT.float32, tag="wev")
                            nc.vector.tensor_copy(out=xe[:, :3], in_=xt[:, :3])
                            nc.sync.dma_start(out=out_x[r0:r0 + 128, :], in_=xe[:, :3])

                # ---- edge update phase ----
                if ci in (1, 3):
                    for st in range(NST):
                        sl = slice(st * 512, (st + 1) * 512)
                        za_ts, zb_ts = [], []
                        for j in range(4):
                            tcol = st * 4 + j
                            za = gat.tile([128, 128], DT.float32, tag="zg")
                            nc.gpsimd.indirect_dma_start(
                                out=za[:], out_offset=None, in_=TAB_ZA[:, :],
                                in_offset=bass.IndirectOffsetOnAxis(
                                    ap=srci[:, tcol:tcol + 1], axis=0))
                            za_ts.append(za)
                            zb = gat.tile([128, 128], DT.float32, tag="zg2")
                            nc.gpsimd.indirect_dma_start(
                                out=zb[:], out_offset=None, in_=TAB_ZB[:, :],
                                in_offset=bass.IndirectOffsetOnAxis(
                                    ap=dsti[:, tcol:tcol + 1], axis=0))
                            zb_ts.append(zb)
                        ef_sb = sb2.tile([128, 512], DT.float32, tag="ef")
                        nc.sync.dma_start(out=ef_sb[:], in_=TAB_EF[:, sl])
                        lin = ps.tile([128, 512], DT.float32, tag="lin")
                        MM(out=lin[:], lhsT=W('euC'), rhs=ef_sb[:],
                           start=True, stop=False, skip_group_check=True)
                        for j in range(4):
                            MM(out=lin[:, j * 128:(j + 1) * 128], lhsT=za_ts[j][:],
                               rhs=ident[:], is_transpose=True, start=False, stop=False,
                               skip_group_check=True)
                            MM(out=lin[:, j * 128:(j + 1) * 128], lhsT=zb_ts[j][:],
                               rhs=ident[:], is_transpose=True, start=False,
                               stop=(j == 3), skip_group_check=True)
                        h1 = sb.tile([128, 512], DT.float32, tag="h1")
                        nc.scalar.activation(out=h1[:], in_=lin[:], func=AF.Silu,
                                             bias=W('eub1')[:, :])
                        p2 = ps.tile([128, 512], DT.float32, tag="lin")
                        MM(out=p2[:], lhsT=W('eu_l2W'), rhs=h1[:], start=True, stop=True)
                        h2 = sb.tile([128, 512], DT.float32, tag="h2")
                        nc.scalar.activation(out=h2[:], in_=p2[:], func=AF.Silu,
                                             bias=W('eu_l2b')[:, :])
                        pre = sb.tile([128, 512], DT.float32, tag="pre")
                        nc.vector.tensor_add(out=pre[:], in0=ef_sb[:], in1=h2[:])
                        ef_new = sb.tile([128, 512], DT.float32, tag="efout")
                        ln_cols(pre[:], 'eu_lng', 'eu_lnb', ef_new[:])
                        nc.sync.dma_start(out=TAB_EF[:, sl], in_=ef_new[:])

                        if ci == 1:
                            # recompute x_diff / d for these 4 tiles
                            for j in range(4):
                                tcol = st * 4 + j
                                esl = slice(tcol * 128, (tcol + 1) * 128)
                                xs = gat.tile([128, 3], DT.float32, tag="xs")
                                nc.gpsimd.indirect_dma_start(
                                    out=xs[:], out_offset=None, in_=TAB_X[:, :],
                                    in_offset=bass.IndirectOffsetOnAxis(
                                        ap=srci[:, tcol:tcol + 1], axis=0))
                                xd = gat.tile([128, 3], DT.float32, tag="xd")
                                nc.gpsimd.indirect_dma_start(
                                    out=xd[:], out_offset=None, in_=TAB_X[:, :],
                                    in_offset=bass.IndirectOffsetOnAxis(
                                        ap=dsti[:, tcol:tcol + 1], axis=0))
                                df = sb.tile([128, 3], DT.float32, tag="df")
                                nc.vector.tensor_sub(out=df[:], in0=xs[:], in1=xd[:])
                                sq2 = sb.tile([128, 3], DT.float32, tag="dsq")
                                nc.vector.tensor_mul(out=sq2[:], in0=df[:], in1=df[:])
                                ss = sb.tile([128, 1], DT.float32, tag="dss")
                                nc.vector.tensor_reduce(
                                    out=ss[:], in_=sq2[:], op=ALU.add,
                                    axis=mybir.AxisListType.X)
                                dij = sb.tile([128, 1], DT.float32, tag="dij")
                                nc.scalar.activation(out=dij[:], in_=ss[:], func=AF.Sqrt,
                                                     bias=W('eps8')[:, :])
                                nc.vector.tensor_scalar_add(out=dij[:], in0=dij[:],
                                                            scalar1=1e-8)
                                inv = sb.tile([128, 1], DT.float32, tag="inv")
                                nc.vector.reciprocal(out=inv[:], in_=dij[:])
                                xdf_et = sb.tile([128, 3], DT.float32, tag="xdfe")
                                nc.vector.tensor_mul(out=xdf_et[:], in0=df[:],
                                                     in1=inv[:].to_broadcast([128, 3]))
                                # rbf: exp(-((dij-mu)/sigma)^2)
                                dmu = sb.tile([128, 16], DT.float32, tag="dmu")
                                nc.vector.tensor_sub(
                                    out=dmu[:], in0=dij[:].to_broadcast([128, 16]),
                                    in1=W('mu16'))
                                sigma = RBF_DMAX / RBF
                                nc.scalar.activation(out=dmu[:], in_=dmu[:],
                                                     func=AF.Square, scale=1.0 / sigma)
                                d_et = sb.tile([128, 16], DT.float32, tag="det")
                                nc.scalar.activation(out=d_et[:], in_=dmu[:],
                                                     func=AF.Exp, scale=-1.0)
                                # transposes back into aux
                                tp = ps.tile([128, 128], DT.float32, tag="wtp")
                                MM(out=tp[:16, :], lhsT=d_et[:], rhs=ident[:],
                                   is_transpose=True, start=True, stop=True)
                                ev16 = sb.tile([16, 128], DT.float32, tag="ev16")
                                nc.vector.tensor_copy(out=ev16[:], in_=tp[:16, :])
                                nc.sync.dma_start(out=AUXD2[0:16, esl], in_=ev16[:])
                                tp2 = ps.tile([128, 128], DT.float32, tag="wtp")
                                MM(out=tp2[:3, :], lhsT=xdf_et[:], rhs=ident[:],
                                   is_transpose=True, start=True, stop=True)
                                ev3 = sb.tile([3, 128], DT.float32, tag="ev3")
                                nc.vector.tensor_copy(out=ev3[:], in_=tp2[:3, :])
                                nc.sync.dma_start(out=AUXD2[32:35, esl], in_=ev3[:])

            # ---------- edge head ----------
            for st in range(NST):
                sl = slice(st * 512, (st + 1) * 512)
                ef_sb = sb2.tile([128, 512], DT.float32, tag="ef")
                nc.sync.dma_start(out=ef_sb[:], in_=TAB_EF[:, sl])
                for j in range(4):
                    tp = ps.tile([128, 128], DT.float32, tag="wtp")
                    MM(out=tp[:], lhsT=ef_sb[:, j * 128:(j + 1) * 128], rhs=ident[:],
                       is_transpose=True, start=True, stop=True)
                    ev = sb.tile([128, 128], DT.float32, tag="wev")
                    nc.vector.tensor_copy(out=ev[:], in_=tp[:])
                    r0 = st * 512 + j * 128
                    nc.sync.dma_start(out=TAB_EFM[r0:r0 + 128, :], in_=ev[:])
            for ht in range(NROWS // 128):
                ue = gat.tile([128, 128], DT.float32, tag="ue")
                nc.gpsimd.indirect_dma_start(
                    out=ue[:], out_offset=None, in_=TAB_EFM[:, :],
                    in_offset=bass.IndirectOffsetOnAxis(ap=Ui[:, ht:ht + 1], axis=0))
                le = gat.tile([128, 128], DT.float32, tag="le")
                nc.gpsimd.indirect_dma_start(
                    out=le[:], out_offset=None, in_=TAB_EFM[:, :],
                    in_offset=bass.IndirectOffsetOnAxis(ap=Mi[:, ht:ht + 1], axis=0))
                ul = sb.tile([128, 128], DT.float32, tag="ul")
                nc.vector.tensor_add(out=ul[:], in0=ue[:], in1=le[:])
                tp = ps.tile([128, 128], DT.float32, tag="wtp")
                MM(out=tp[:], lhsT=ul[:], rhs=ident[:], is_transpose=True,
                   start=True, stop=True)
                ulT = sb.tile([128, 128], DT.float32, tag="ulT")
                nc.vector.tensor_copy(out=ulT[:], in_=tp[:])
                hp = ps.tile([128, 128], DT.float32, tag="lin")
                MM(out=hp[:], lhsT=W('eh_l1W'), rhs=ulT[:], start=True, stop=True)
                hh = sb.tile([128, 128], DT.float32, tag="ehh")
                nc.scalar.activation(out=hh[:], in_=hp[:], func=AF.Silu,
                                     bias=W('eh_l1b')[:, :])
                op = ps.tile([5, 128], DT.float32, tag="wtp")
                MM(out=op[:], lhsT=W('eh_l2W'), rhs=hh[:], start=True, stop=True)
                ob = sb.tile([5, 128], DT.float32, tag="ehob")
                nc.scalar.activation(out=ob[:], in_=op[:], func=AF.Identity,
                                     bias=W('eh_l2b')[:, :])
                tp2 = ps.tile([128, 128], DT.float32, tag="wtp")
                MM(out=tp2[:, :5], lhsT=ob[:], rhs=ident[:5, :5], is_transpose=True,
                   start=True, stop=True)
                ev = sb.tile([128, 128], DT.float32, tag="wev")
                nc.vector.tensor_copy(out=ev[:, :5], in_=tp2[:, :5])
                nc.sync.dma_start(out=out_el[ht * 128:(ht + 1) * 128, :], in_=ev[:, :5])

    nc.compile()
    return nc


# ================= entry point =================

def kernel(**inputs):
    from concourse import bass_utils

    pk = pack_weights(inputs['params'])
    B = build_blob(pk)
    wb_arr = B.finalize()
    cores, s0T, x0T = prep(inputs)

    key = ('prog', wb_arr.shape[1])
    if key not in _CACHE:
        _CACHE[key] = build_program(B.entries, wb_arr.shape[1])
    nc = _CACHE[key]

    in_maps = []
    for c in range(NC8):
        co = cores[c]
        in_maps.append({
            'WB': wb_arr, 's0T': s0T, 'x0T': x0T, 'aux0': co['aux0'],
            'dstf': co['dstf'],
            'srci': co['srci'], 'dsti': co['dsti'], 'Ui': co['Ui'], 'Mi': co['Mi'],
            'e0T': co['e0T'],
        })
    res = bass_utils.run_bass_kernel_spmd(nc, in_maps, core_ids=list(range(NC8)))
    _CACHE['last_exec_ns'] = res.exec_time_ns
    _CACHE['last_results'] = res.results

    r0 = res.results[0]
    x = r0['out_x'][:N].astype(F32)
    a_logits = r0['out_nh'][:N, :10].astype(F32)
    c_logits = r0['out_nh'][:N, 10:16].astype(F32)
    e_logits = np.concatenate(
        [res.results[c]['out_el'][:EC // 2] for c in range(NC8)], 0).astype(F32)
    return x, a_logits, c_logits, e_logits


# revision 33
# speedup vs baseline: 45.0444x; 45.0444x over previous
"""Trainium2 Bass kernel for nn_EndpointVectorField (GVP message-passing GNN).

Strategy (8 NeuronCores, SPMD):
  - Edges sharded by pair: core c owns upper edges [c*10k,(c+1)*10k) and their
    reverse mates, sorted by dst and packed into 128-edge tiles that never split
    a dst group (enables race-free scatter via selection-matrix matmul).
  - All activations feature-on-partition [128, cols]; node tables in DRAM are
    gathered per edge via indirect DMA and transposed on the PE.
  - Per-node aggregation: per-core partial tables scattered to DRAM, AllReduce
    across cores, node update replicated on every core.
  - Vector channel (GVP) path packed as block-diagonal matmuls over the 3
    coords with Wh and Wh@Wu fused side by side.
"""
import sys
import numpy as np

if '/opt/trn_rl_repo' not in sys.path:
    sys.path.insert(0, '/opt/trn_rl_repo')

# ---- problem constants (hardcoded per contract) ----
N = 10000
EH = 80000
E = 2 * EH
HS = 128
RBF = 16
RBF_DMAX = 20.0
NC8 = 8
EC = E // NC8          # 20000 edges/core
NT = 164               # 128-edge tiles per core
EP = NT * 128          # 20992 padded edges
NST = NT // 4          # 41 super-tiles of 512
NROWS = 10240          # padded node rows
NNT = NROWS // 512     # 20 node super-tiles
TRASH = N
LN_EPS = 1e-5
F32 = np.float32

_CACHE = {}


def f32(x):
    return np.asarray(x, dtype=F32)


# ================= weight packing (validated in np sim) =================

def pack_gvp(p, vi_has_extra, dh, vo):
    Wh = f32(p['Wh']); Wu = f32(p['Wu'])
    WhWu = f32(Wh @ Wu)
    v_off = 1 if vi_has_extra else 0
    Wvh = np.zeros((48, 3 * dh), F32)
    Wvu = np.zeros((48, 3 * vo), F32)
    for c in range(3):
        for v in range(16):
            Wvh[c * 16 + v, c * dh:(c + 1) * dh] = Wh[v_off + v]
            Wvu[c * 16 + v, c * vo:(c + 1) * vo] = WhWu[v_off + v]
    out = {'Wvh': Wvh, 'Wvu': Wvu}
    if vi_has_extra:
        Wvh_x = np.zeros((3, 3 * dh), F32)
        Wvu_x = np.zeros((3, 3 * vo), F32)
        for c in range(3):
            Wvh_x[c, c * dh:(c + 1) * dh] = Wh[0]
            Wvu_x[c, c * vo:(c + 1) * vo] = WhWu[0]
        out['Wvh_x'] = Wvh_x
        out['Wvu_x'] = Wvu_x
    W = f32(p['lin']['W'])
    fi = W.shape[0] - dh
    out['Wlin_s'] = W[:fi]
    out['Wlin_sh'] = W[fi:]
    out['blin'] = f32(p['lin']['b'])
    Wg = f32(p['gate']['W']); bg = f32(p['gate']['b'])
    Wg_rep = np.zeros((Wg.shape[0], 3 * vo), F32)
    bg_rep = np.zeros((3 * vo,), F32)
    for c in range(3):
        Wg_rep[:, c * vo:(c + 1) * vo] = Wg
        bg_rep[c * vo:(c + 1) * vo] = bg
    out['Wg'] = Wg_rep
    out['bg'] = bg_rep
    out['dh'] = dh; out['vo'] = vo
    return out


def pack_weights(params):
    pk = {}
    for nm, src in [('scal_l1', params['scal_emb']['l1']),
                    ('scal_l2', params['scal_emb']['l2']),
                    ('edge_l1', params['edge_emb']['l1']),
                    ('edge_l2', params['edge_emb']['l2']),
                    ('eu_l2', params['edge_upd']['l2']),
                    ('nh_l1', params['node_head']['l1']),
                    ('nh_l2', params['node_head']['l2']),
                    ('eh_l1', params['edge_head']['l1']),
                    ('eh_l2', params['edge_head']['l2'])]:
        pk[nm] = (f32(src['W']), f32(src['b']))
    for nm, src in [('scal_ln', params['scal_emb']['ln']),
                    ('edge_ln', params['edge_emb']['ln']),
                    ('eu_ln', params['edge_upd']['ln'])]:
        pk[nm] = (f32(src['g']), f32(src['b']))
    pk['convs'] = []
    for ci in range(4):
        cv = params['convs'][ci]
        msg1 = pack_gvp(cv['msg'][0], True, 17, 16)
        W = msg1['Wlin_s']
        msg1['Ws'] = W[:HS].copy()
        msg1['Wd'] = W[HS:HS + RBF].copy()
        msg1['Wef'] = W[HS + RBF:].copy()
        msg = [msg1, pack_gvp(cv['msg'][1], False, 16, 16),
               pack_gvp(cv['msg'][2], False, 16, 16)]
        upd = [pack_gvp(cv['upd'][k], False, 16, 16) for k in range(3)]
        pk['convs'].append({
            'msg': msg, 'upd': upd,
            'ln_msg': (f32(cv['ln_msg']['g']), f32(cv['ln_msg']['b'])),
            'ln_upd': (f32(cv['ln_upd']['g']), f32(cv['ln_upd']['b']))})
    pk['pos'] = [pack_gvp(params['pos_upd'][0], False, 16, 16),
                 pack_gvp(params['pos_upd'][1], False, 16, 16),
                 pack_gvp(params['pos_upd'][2], False, 16, 1)]
    eu = params['edge_upd']
    W1 = f32(eu['l1']['W'])
    pk['eu_A'] = W1[:HS].copy()
    pk['eu_B'] = W1[HS:2 * HS].copy()
    pk['eu_C'] = W1[2 * HS:].copy()
    pk['eu_b1'] = f32(eu['l1']['b'])
    return pk


class Blob:
    """Packs 2-D f32 matrices into one [128, cols] SBUF-resident blob."""

    def __init__(self):
        self.cols = 0
        self.entries = {}   # name -> (row0, K, col0, M)
        self.arrays = {}

    def add(self, name, arr, row0=0):
        arr = f32(arr)
        if arr.ndim == 1:
            arr = arr[:, None]
        K, M = arr.shape
        assert row0 + K <= 128
        self.entries[name] = (row0, K, self.cols, M)
        self.arrays[name] = arr
        self.cols += M
        return name

    def finalize(self):
        buf = np.zeros((128, self.cols), F32)
        for name, (r0, K, c0, M) in self.entries.items():
            buf[r0:r0 + K, c0:c0 + M] = self.arrays[name]
        return buf


def build_blob(pk):
    B = Blob()
    B.add('ones128', np.ones((128, 1), F32))
    B.add('ones1', np.ones((1, 128), F32))
    ssel17 = np.zeros((51, 17), F32)
    for c in range(3):
        ssel17[c * 17:(c + 1) * 17] = np.eye(17, dtype=F32)
    B.add('ssel17', ssel17)
    ssel16 = np.zeros((48, 16), F32)
    for c in range(3):
        ssel16[c * 16:(c + 1) * 16] = np.eye(16, dtype=F32)
    B.add('ssel16', ssel16)
    mu = np.linspace(0.0, RBF_DMAX, RBF, dtype=F32)
    B.add('mu16', np.broadcast_to(mu[None, :], (128, RBF)).copy())
    B.add('eps8', np.full((128, 1), 1e-8, F32))
    B.add('epsln', np.full((128, 1), LN_EPS, F32))

    def add_gvp(pref, g):
        B.add(pref + 'Wvh', g['Wvh'])
        B.add(pref + 'Wvu', g['Wvu'])
        if 'Wvh_x' in g:
            B.add(pref + 'Wvh_x', g['Wvh_x'], row0=32)
            B.add(pref + 'Wvu_x', g['Wvu_x'], row0=32)
        if 'Ws' not in g and g['Wlin_s'].shape[0] in (128, 16):
            B.add(pref + 'Wls', g['Wlin_s'])
        B.add(pref + 'Wlsh', g['Wlin_sh'])
        B.add(pref + 'blin', g['blin'])
        B.add(pref + 'Wg', g['Wg'])
        B.add(pref + 'bg', g['bg'])

    for ci in range(4):
        cv = pk['convs'][ci]
        m1 = cv['msg'][0]
        B.add(f'c{ci}z1w', m1['Ws'])
        B.add(f'c{ci}m1Wd', m1['Wd'])
        B.add(f'c{ci}m1Wef', m1['Wef'])
        add_gvp(f'c{ci}m1', m1)
        add_gvp(f'c{ci}m2', cv['msg'][1])
        add_gvp(f'c{ci}m3', cv['msg'][2])
        for k in range(3):
            add_gvp(f'c{ci}u{k}', cv['upd'][k])
        B.add(f'c{ci}lnmg', cv['ln_msg'][0])
        B.add(f'c{ci}lnmb', cv['ln_msg'][1])
        B.add(f'c{ci}lnug', cv['ln_upd'][0])
        B.add(f'c{ci}lnub', cv['ln_upd'][1])
    for k in range(3):
        add_gvp(f'p{k}', pk['pos'][k])
    B.add('euA', pk['eu_A'])
    B.add('euB', pk['eu_B'])
    B.add('euC', pk['eu_C'])
    B.add('eub1', pk['eu_b1'])
    for nm in ['eu_l2', 'nh_l1', 'nh_l2', 'eh_l1', 'eh_l2',
               'scal_l1', 'scal_l2', 'edge_l1', 'edge_l2']:
        B.add(nm + 'W', pk[nm][0])
        B.add(nm + 'b', pk[nm][1])
    for nm in ['scal_ln', 'edge_ln', 'eu_ln']:
        B.add(nm + 'g', pk[nm][0])
        B.add(nm + 'b', pk[nm][1])
    return B


# ================= host prep =================

def prep(inputs):
    src = np.asarray(inputs['src_idx']).astype(np.int64)
    dst = np.asarray(inputs['dst_idx']).astype(np.int64)
    e_t = f32(inputs['e_t'])
    a_t = f32(inputs['a_t']); c_t = f32(inputs['c_t'])
    x_t = f32(inputs['x_t']); t = f32(inputs['t'])
    nbi = np.asarray(inputs['node_batch_idx']).astype(np.int64)

    s0 = np.zeros((N, 17), F32)
    s0[:, :10] = a_t
    s0[:, 10] = t[nbi]
    s0[:, 11:] = c_t
    s0T = np.zeros((17, NROWS), F32)
    s0T[:, :N] = s0.T
    x0T = np.zeros((3, NROWS), F32)
    x0T[:, :N] = x_t.T

    mu = np.linspace(0.0, RBF_DMAX, RBF, dtype=F32)
    sigma = F32(RBF_DMAX / RBF)

    cores = []
    for c in range(NC8):
        PH = EC // 2
        gidx = np.concatenate([np.arange(c * PH, (c + 1) * PH),
                               EH + np.arange(c * PH, (c + 1) * PH)])
        sc = src[gidx]; dc = dst[gidx]
        order = np.argsort(dc, kind='stable')
        ds = dc[order]
        groups = []
        run = 0
        for i in range(1, len(ds) + 1):
            if i == len(ds) or ds[i] != ds[i - 1]:
                groups.append((run, i)); run = i
        tiles = []
        cur = []; cur_len = 0
        for (a, b) in groups:
            gl = b - a
            assert gl <= 128, f"in-degree {gl} > 128"
            if cur_len + gl > 128:
                tiles.append(np.concatenate(cur)); cur = []; cur_len = 0
            cur.append(order[a:b]); cur_len += gl
        if cur_len:
            tiles.append(np.concatenate(cur))
        assert len(tiles) <= NT, f"core {c}: {len(tiles)} tiles > {NT}"
        src_p = np.zeros(EP, np.int32)
        dst_p = np.full(EP, TRASH, np.int32)
        e0_p = np.zeros((EP, 5), F32)
        gid_p = np.full(EP, -1, np.int64)
        for ti, tl in enumerate(tiles):
            n = len(tl)
            src_p[ti * 128:ti * 128 + n] = sc[tl]
            dst_p[ti * 128:ti * 128 + n] = dc[tl]
            e0_p[ti * 128:ti * 128 + n] = e_t[gidx[tl]]
            gid_p[ti * 128:ti * 128 + n] = gidx[tl]
        pos_of = {}
        for p_, g_ in enumerate(gid_p):
            if g_ >= 0:
                pos_of[g_] = p_
        up = np.arange(c * PH, (c + 1) * PH)
        U = np.zeros(NROWS, np.int32)
        M = np.zeros(NROWS, np.int32)
        U[:PH] = [pos_of[g_] for g_ in up]
        M[:PH] = [pos_of[g_ + EH] for g_ in up]

        diff = (x_t[src_p] - np.where((dst_p < N)[:, None],
                                      x_t[np.minimum(dst_p, N - 1)], 0.0)).astype(F32)
        dij = (np.sqrt((diff * diff).sum(1) + F32(1e-8)) + F32(1e-8)).astype(F32)
        xdf = (diff / dij[:, None]).astype(F32)
        d0 = np.exp(-(((dij[:, None] - mu[None, :]) / sigma) ** 2)).astype(F32)

        aux0 = np.zeros((35, EP), F32)
        aux0[0:16] = d0.T
        aux0[32:35] = xdf.T

        cores.append({
            'aux0': aux0,
            'dstf': dst_p.astype(F32).reshape(NT, 128).T.copy(),
            'srci': src_p.reshape(NT, 128).T.copy(),
            'dsti': dst_p.reshape(NT, 128).T.copy(),
            'Ui': U.reshape(NROWS // 128, 128).T.copy(),
            'Mi': M.reshape(NROWS // 128, 128).T.copy(),
            'e0T': np.ascontiguousarray(e0_p.T),
        })
    return cores, s0T, x0T


# ================= device program =================

def build_program(blob_entries, wcols):
    import concourse.bass as bass
    import concourse.bacc as bacc
    import concourse.tile as tile
    from concourse import mybir
    from concourse.masks import make_identity

    AF = mybir.ActivationFunctionType
    ALU = mybir.AluOpType
    DT = mybir.dt

    nc = bacc.Bacc("TRN2", target_bir_lowering=False, debug=False,
                   num_devices=NC8, enable_asserts=False)

    # ---- I/O ----
    WB = nc.dram_tensor("WB", [128, wcols], DT.float32, kind="ExternalInput")
    s0T = nc.dram_tensor("s0T", [17, NROWS], DT.float32, kind="ExternalInput")
    aux0 = nc.dram_tensor("aux0", [35, EP], DT.float32, kind="ExternalInput")
    x0T_in = nc.dram_tensor("x0T", [3, NROWS], DT.float32, kind="ExternalInput")
    dstf_in = nc.dram_tensor("dstf", [128, NT], DT.float32, kind="ExternalInput")
    srci_in = nc.dram_tensor("srci", [128, NT], DT.int32, kind="ExternalInput")
    dsti_in = nc.dram_tensor("dsti", [128, NT], DT.int32, kind="ExternalInput")
    Ui_in = nc.dram_tensor("Ui", [128, NROWS // 128], DT.int32, kind="ExternalInput")
    Mi_in = nc.dram_tensor("Mi", [128, NROWS // 128], DT.int32, kind="ExternalInput")
    e0T_in = nc.dram_tensor("e0T", [5, EP], DT.float32, kind="ExternalInput")

    out_x = nc.dram_tensor("out_x", [NROWS, 3], DT.float32, kind="ExternalOutput")
    out_nh = nc.dram_tensor("out_nh", [NROWS, 16], DT.float32, kind="ExternalOutput")
    out_el = nc.dram_tensor("out_el", [NROWS, 5], DT.float32, kind="ExternalOutput")

    # ---- internal DRAM tables ----
    TAB_Z1 = nc.dram_tensor("TAB_Z1", [NROWS, 128], DT.float32, kind="Internal")
    TAB_V = nc.dram_tensor("TAB_V", [NROWS, 48], DT.float32, kind="Internal")
    TAB_ZA = nc.dram_tensor("TAB_ZA", [NROWS, 128], DT.float32, kind="Internal")
    TAB_ZB = nc.dram_tensor("TAB_ZB", [NROWS, 128], DT.float32, kind="Internal")
    TAB_X = nc.dram_tensor("TAB_X", [NROWS, 3], DT.float32, kind="Internal")
    TAB_S = nc.dram_tensor("TAB_S", [128, NROWS], DT.float32, kind="Internal")
    TAB_VF = nc.dram_tensor("TAB_VF", [48, NROWS], DT.float32, kind="Internal")
    TAB_EF = nc.dram_tensor("TAB_EF", [128, EP], DT.float32, kind="Internal")
    TAB_EFM = nc.dram_tensor("TAB_EFM", [EP, 128], DT.float32, kind="Internal")
    AUXD2 = nc.dram_tensor("AUXD2", [35, EP], DT.float32, kind="Internal")
    XF = nc.dram_tensor("XF", [3, NROWS], DT.float32, kind="Internal")
    AGG_IN = [nc.dram_tensor(f"AGG_IN{ci}", [NROWS, 176], DT.float32, kind="Internal")
              for ci in range(4)]
    AGG_OUT = [nc.dram_tensor(f"AGG_OUT{ci}", [NROWS, 176], DT.float32,
                              kind="Internal", addr_space="Shared")
               for ci in range(4)]

    with tile.TileContext(nc) as tc:
        from contextlib import ExitStack
        ctx = ExitStack()
        with ctx:
            persist = ctx.enter_context(tc.tile_pool(name="persist", bufs=1))
            sb = ctx.enter_context(tc.tile_pool(name="sb", bufs=1))
            sb2 = ctx.enter_context(tc.tile_pool(name="sb2", bufs=2))
            gat = ctx.enter_context(tc.tile_pool(name="gat", bufs=8))
            ps = ctx.enter_context(tc.tile_pool(name="ps", bufs=1, space="PSUM"))

            wb = persist.tile([128, wcols], DT.float32)
            nc.sync.dma_start(out=wb[:], in_=WB[:, :])

            def W(name):
                r0, K, c0, M = blob_entries[name]
                return wb[r0:r0 + K, c0:c0 + M]

            dstf = persist.tile([128, NT], DT.float32)
            srci = persist.tile([128, NT], DT.int32)
            dsti = persist.tile([128, NT], DT.int32)
            Ui = persist.tile([128, NROWS // 128], DT.int32)
            Mi = persist.tile([128, NROWS // 128], DT.int32)
            for t_, i_ in [(dstf, dstf_in), (srci, srci_in), (dsti, dsti_in),
                           (Ui, Ui_in), (Mi, Mi_in)]:
                nc.sync.dma_start(out=t_[:], in_=i_[:, :])

            ident = persist.tile([128, 128], DT.float32)
            make_identity(nc, ident[:])
            zero_sb = persist.tile([128, 176], DT.float32)
            nc.vector.memset(zero_sb[:], 0.0)

            MM = nc.tensor.matmul

            # ---------- helpers ----------
            def ln_cols(pre_sb, gname, bname, out_t, n=512):
                """LayerNorm over partitions for [128, n] tile -> out_t."""
                stp = ps.tile([33, 512], DT.float32, tag="stats")
                stats = stp[0:1, :]
                sqs = stp[32:33, :]
                sq_sb = sb.tile([128, 512], DT.float32, tag="lnw")
                MM(out=stats[:, :n], lhsT=W('ones128'), rhs=pre_sb, start=True, stop=True)
                nc.scalar.activation(out=sq_sb[:, :n], in_=pre_sb, func=AF.Square)
                MM(out=sqs[:, :n], lhsT=W('ones128'), rhs=sq_sb[:, :n], start=True, stop=True)
                st_ = sb.tile([1, 2048], DT.float32, tag="lnst")
                m_sb = st_[0:1, 0:n]
                v_sb = st_[0:1, 512:512 + n]
                m2 = st_[0:1, 1024:1024 + n]
                r_sb = st_[0:1, 1536:1536 + n]
                nc.scalar.activation(out=m_sb, in_=stats[:, :n], func=AF.Copy,
                                     scale=1.0 / 128.0)
                nc.scalar.activation(out=v_sb, in_=sqs[:, :n], func=AF.Copy,
                                     scale=1.0 / 128.0)
                nc.vector.tensor_mul(out=m2, in0=m_sb, in1=m_sb)
                nc.vector.tensor_sub(out=v_sb, in0=v_sb, in1=m2)
                nc.scalar.activation(out=v_sb, in_=v_sb, func=AF.Sqrt,
                                     bias=W('epsln')[0:1, :])
                nc.vector.reciprocal(out=r_sb, in_=v_sb)
                mb = ps.tile([128, 512], DT.float32, tag="scat")
                rb = ps.tile([128, 512], DT.float32, tag="red")
                MM(out=mb[:, :n], lhsT=W('ones1'), rhs=m_sb, start=True, stop=True)
                MM(out=rb[:, :n], lhsT=W('ones1'), rhs=r_sb, start=True, stop=True)
                cs = sb.tile([128, 512], DT.float32, tag="lnw")
                nc.vector.tensor_sub(out=cs[:, :n], in0=pre_sb, in1=mb[:, :n])
                nc.vector.tensor_mul(out=cs[:, :n], in0=cs[:, :n], in1=rb[:, :n])
                nc.scalar.activation(out=out_t, in_=cs[:, :n], func=AF.Identity,
                                     scale=W(gname)[:, :], bias=W(bname)[:, :])

            def gvp(pref, dh, vo, lin_ins, mv_sb_ap, xdf_ap, out_ms, sigmoid_gate=True):
                """One GVP. lin_ins: list of (lhsT_name_or_ap, rhs_ap, K) matmul
                contributions plus optional ('T', src_tile) transpose contribs.
                mv_sb_ap: [48, 512] SBUF. Returns (gate_or_sig_sb, vu_psum)."""
                vh = ps.tile([51, 512], DT.float32, tag="vh")
                vu = ps.tile([48, 512], DT.float32, tag="vu")
                MM(out=vh[:3 * dh, :], lhsT=W(pref + 'Wvh'), rhs=mv_sb_ap,
                   start=True, stop=(xdf_ap is None))
                MM(out=vu[:3 * vo, :], lhsT=W(pref + 'Wvu'), rhs=mv_sb_ap,
                   start=True, stop=(xdf_ap is None))
                if xdf_ap is not None:
                    MM(out=vh[:3 * dh, :], lhsT=W(pref + 'Wvh_x'), rhs=xdf_ap,
                       start=False, stop=True)
                    MM(out=vu[:3 * vo, :], lhsT=W(pref + 'Wvu_x'), rhs=xdf_ap,
                       start=False, stop=True)
                sq = sb2.tile([51, 512], DT.float32, tag="sq")
                nc.scalar.activation(out=sq[:3 * dh, :], in_=vh[:3 * dh, :], func=AF.Square)
                ssq = ps.tile([17, 512], DT.float32, tag="gvaux")
                sselw = 'ssel17' if dh == 17 else 'ssel16'
                MM(out=ssq[:dh, :], lhsT=W(sselw), rhs=sq[:3 * dh, :], start=True, stop=True)
                sh = sb2.tile([17, 512], DT.float32, tag="sh")
                nc.scalar.activation(out=sh[:dh, :], in_=ssq[:dh, :], func=AF.Sqrt,
                                     bias=W('eps8')[0:dh, :])
                lin = ps.tile([128, 512], DT.float32, tag="lin")
                first = True
                for item in lin_ins:
                    if item[0] == 'T':
                        assert not first, "transposes must accumulate after a start"
                        for j, zt in enumerate(item[1]):
                            MM(out=lin[:, j * 128:(j + 1) * 128], lhsT=zt,
                               rhs=ident[:], is_transpose=True,
                               start=False, stop=False, skip_group_check=True)
                    else:
                        lname, rhs_ap = item
                        MM(out=lin[:], lhsT=W(lname), rhs=rhs_ap,
                           start=first, stop=False, skip_group_check=True)
                        first = False
                MM(out=lin[:], lhsT=W(pref + 'Wlsh'), rhs=sh[:dh, :],
                   start=False, stop=True, skip_group_check=True)
                nc.scalar.activation(out=out_ms, in_=lin[:], func=AF.Silu,
                                     bias=W(pref + 'blin')[:, :])
                gate = ps.tile([48, 512], DT.float32, tag="gvaux")
                MM(out=gate[:3 * vo, :], lhsT=W(pref + 'Wg'), rhs=out_ms,
                   start=True, stop=True)
                sig = sb2.tile([48, 512], DT.float32, tag="sig")
                nc.scalar.activation(out=sig[:3 * vo, :], in_=gate[:3 * vo, :],
                                     func=AF.Sigmoid if sigmoid_gate else AF.Identity,
                                     bias=W(pref + 'bg')[:3 * vo, :])
                return sig, vu

            def gvp_chain2(pref2, pref3, ms_in, mv_in, out_ms, out_mv):
                """GVP2 then GVP3 (dh=vo=16), edge or node side."""
                ms2 = sb2.tile([128, 512], DT.float32, tag="ms2")
                sig2, vu2 = gvp(pref2, 16, 16, [(pref2 + 'Wls', ms_in)], mv_in, None, ms2[:])
                mv2 = sb2.tile([48, 512], DT.float32, tag="mv2")
                nc.vector.tensor_mul(out=mv2[:], in0=sig2[:], in1=vu2[:])
                sig3, vu3 = gvp(pref3, 16, 16, [(pref3 + 'Wls', ms2[:])], mv2[:], None, out_ms)
                nc.vector.tensor_mul(out=out_mv, in0=sig3[:], in1=vu3[:])

            def write_nodemaj(src_sb, tab, nt, width):
                """[width<=128, 512] feature-major tile -> node-major DRAM rows."""
                for j in range(4):
                    tp = ps.tile([128, 128], DT.float32, tag="wtp")
                    MM(out=tp[:, :width], lhsT=src_sb[:, j * 128:(j + 1) * 128],
                       rhs=ident[:width, :width], is_transpose=True, start=True, stop=True)
                    ev = sb.tile([128, 128], DT.float32, tag="wev")
                    nc.vector.tensor_copy(out=ev[:, :width], in_=tp[:, :width])
                    nc.sync.dma_start(
                        out=tab[nt * 512 + j * 128: nt * 512 + (j + 1) * 128, :],
                        in_=ev[:, :width])

            # ---------- init node phase: scal_emb ----------
            for nt in range(NNT):
                sl = slice(nt * 512, (nt + 1) * 512)
                s0_sb = sb.tile([17, 512], DT.float32, tag="s0")
                nc.sync.dma_start(out=s0_sb[:], in_=s0T[:, sl])
                p1 = ps.tile([128, 512], DT.float32, tag="lin")
                MM(out=p1[:], lhsT=W('scal_l1W'), rhs=s0_sb[:], start=True, stop=True)
                h1 = sb.tile([128, 512], DT.float32, tag="h1")
                nc.scalar.activation(out=h1[:], in_=p1[:], func=AF.Silu,
                                     bias=W('scal_l1b')[:, :])
                p2 = ps.tile([128, 512], DT.float32, tag="lin")
                MM(out=p2[:], lhsT=W('scal_l2W'), rhs=h1[:], start=True, stop=True)
                h2 = sb.tile([128, 512], DT.float32, tag="h2")
                nc.scalar.activation(out=h2[:], in_=p2[:], func=AF.Silu,
                                     bias=W('scal_l2b')[:, :])
                s_sb = sb.tile([128, 512], DT.float32, tag="sout")
                ln_cols(h2[:], 'scal_lng', 'scal_lnb', s_sb[:])
                nc.sync.dma_start(out=TAB_S[:, sl], in_=s_sb[:])
                z1 = ps.tile([128, 512], DT.float32, tag="lin")
                MM(out=z1[:], lhsT=W('c0z1w'), rhs=s_sb[:], start=True, stop=True)
                z1s = sb.tile([128, 512], DT.float32, tag="z1s")
                nc.vector.tensor_copy(out=z1s[:], in_=z1[:])
                write_nodemaj(z1s[:], TAB_Z1, nt, 128)
                # zero V tables + TAB_X / XF init
                xb = sb.tile([3, 512], DT.float32, tag="xb")
                nc.sync.dma_start(out=xb[:], in_=x0T_in[:, sl])
                nc.sync.dma_start(out=XF[:, sl], in_=xb[:])
                for j in range(4):
                    r0 = nt * 512 + j * 128
                    nc.sync.dma_start(out=TAB_V[r0:r0 + 128, :], in_=zero_sb[:, :48])
                    xt = ps.tile([128, 128], DT.float32, tag="wtp")
                    MM(out=xt[:, :3], lhsT=xb[:, j * 128:(j + 1) * 128],
                       rhs=ident[0:3, 0:3], is_transpose=True, start=True, stop=True)
                    xe = sb.tile([128, 128], DT.float32, tag="wev")
                    nc.vector.tensor_copy(out=xe[:, :3], in_=xt[:, :3])
                    nc.sync.dma_start(out=TAB_X[r0:r0 + 128, :], in_=xe[:, :3])
                vz = sb2.tile([48, 512], DT.float32, tag="mv3")
                nc.vector.memset(vz[:], 0.0)
                nc.sync.dma_start(out=TAB_VF[:, sl], in_=vz[:])

            # ---------- init edge phase: edge_emb ----------
            for st in range(NST):
                sl = slice(st * 512, (st + 1) * 512)
                e0_sb = sb.tile([5, 512], DT.float32, tag="e0")
                nc.sync.dma_start(out=e0_sb[:], in_=e0T_in[:, sl])
                p1 = ps.tile([128, 512], DT.float32, tag="lin")
                MM(out=p1[:], lhsT=W('edge_l1W'), rhs=e0_sb[:], start=True, stop=True)
                h1 = sb.tile([128, 512], DT.float32, tag="h1")
                nc.scalar.activation(out=h1[:], in_=p1[:], func=AF.Silu,
                                     bias=W('edge_l1b')[:, :])
                p2 = ps.tile([128, 512], DT.float32, tag="lin")
                MM(out=p2[:], lhsT=W('edge_l2W'), rhs=h1[:], start=True, stop=True)
                h2 = sb.tile([128, 512], DT.float32, tag="h2")
                nc.scalar.activation(out=h2[:], in_=p2[:], func=AF.Silu,
                                     bias=W('edge_l2b')[:, :])
                ef_sb = sb.tile([128, 512], DT.float32, tag="efout")
                ln_cols(h2[:], 'edge_lng', 'edge_lnb', ef_sb[:])
                nc.sync.dma_start(out=TAB_EF[:, sl], in_=ef_sb[:])

            # ---------- conv loop ----------
            for ci in range(4):
                agg_in = AGG_IN[ci]
                agg_out = AGG_OUT[ci]
                # zero the partial table
                for r in range(NROWS // 128):
                    nc.sync.dma_start(out=agg_in[r * 128:(r + 1) * 128, :],
                                      in_=zero_sb[:, :])

                # ---- edge message phase ----
                for st in range(NST):
                    sl = slice(st * 512, (st + 1) * 512)
                    zts = []
                    vg_ts = []
                    for j in range(4):
                        tcol = st * 4 + j
                        zg = gat.tile([128, 128], DT.float32, tag="zg")
                        nc.gpsimd.indirect_dma_start(
                            out=zg[:], out_offset=None, in_=TAB_Z1[:, :],
                            in_offset=bass.IndirectOffsetOnAxis(
                                ap=srci[:, tcol:tcol + 1], axis=0))
                        zts.append(zg)
                        vg = gat.tile([128, 48], DT.float32, tag="vg")
                        nc.gpsimd.indirect_dma_start(
                            out=vg[:], out_offset=None, in_=TAB_V[:, :],
                            in_offset=bass.IndirectOffsetOnAxis(
                                ap=srci[:, tcol:tcol + 1], axis=0))
                        vg_ts.append(vg)
                    ef_sb = sb2.tile([128, 512], DT.float32, tag="ef")
                    nc.sync.dma_start(out=ef_sb[:], in_=TAB_EF[:, sl])
                    dxf = sb2.tile([35, 512], DT.float32, tag="dxf")
                    nc.sync.dma_start(out=dxf[:],
                                      in_=(aux0 if ci < 2 else AUXD2)[:, sl])
                    # V transpose -> mv0 [48, 512]
                    vtp = ps.tile([48, 512], DT.float32, tag="gvaux")
                    for j in range(4):
                        MM(out=vtp[:, j * 128:(j + 1) * 128], lhsT=vg_ts[j][:],
                           rhs=ident[:], is_transpose=True, start=True, stop=True)
                    mv0 = sb2.tile([48, 512], DT.float32, tag="mv0")
                    nc.vector.tensor_copy(out=mv0[:], in_=vtp[:])

                    ms1 = sb2.tile([128, 512], DT.float32, tag="ms1")
                    sig1, vu1 = gvp(
                        f'c{ci}m1', 17, 16,
                        [(f'c{ci}m1Wef', ef_sb[:]),
                         (f'c{ci}m1Wd', dxf[0:16, :]),
                         ('T', [z[:] for z in zts])],
                        mv0[:], dxf[32:35, :], ms1[:])
                    mv1 = sb2.tile([48, 512], DT.float32, tag="mv1")
                    nc.vector.tensor_mul(out=mv1[:], in0=sig1[:], in1=vu1[:])
                    ms3 = sb2.tile([128, 512], DT.float32, tag="ms3")
                    mv3 = sb2.tile([48, 512], DT.float32, tag="mv3")
                    gvp_chain2(f'c{ci}m2', f'c{ci}m3', ms1[:], mv1[:], ms3[:], mv3[:])

                    # ---- scatter ----
                    for j in range(4):
                        tcol = st * 4 + j
                        sp = ps.tile([128, 512], DT.float32, tag="scat")
                        # dstT
                        MM(out=sp[:, 0:128],
                           lhsT=dstf[:, tcol:tcol + 1].to_broadcast([128, 128]),
                           rhs=ident[:], is_transpose=True, start=True, stop=True)
                        dstT = sb.tile([128, 128], DT.float32, tag="dstT")
                        nc.vector.tensor_copy(out=dstT[:], in_=sp[:, 0:128])
                        sel = sb.tile([128, 128], DT.float32, tag="sel")
                        nc.vector.tensor_tensor(
                            out=sel[:],
                            in0=dstf[:, tcol:tcol + 1].to_broadcast([128, 128]),
                            in1=dstT[:], op=ALU.is_equal)
                        # ms/mv transposes
                        MM(out=sp[:, 128:256], lhsT=ms3[:, j * 128:(j + 1) * 128],
                           rhs=ident[:], is_transpose=True, start=True, stop=True)
                        MM(out=sp[:, 256:304], lhsT=mv3[:, j * 128:(j + 1) * 128],
                           rhs=ident[:48, :48], is_transpose=True, start=True, stop=True)
                        ets = sb.tile([128, 176], DT.float32, tag="ets")
                        nc.vector.tensor_copy(out=ets[:], in_=sp[:, 128:304])
                        red = ps.tile([128, 176], DT.float32, tag="red")
                        MM(out=red[:, 0:128], lhsT=sel[:], rhs=ets[:, 0:128],
                           start=True, stop=True)
                        MM(out=red[:, 128:176], lhsT=sel[:], rhs=ets[:, 128:176],
                           start=True, stop=True)
                        redsb = sb.tile([128, 176], DT.float32, tag="redsb")
                        nc.scalar.activation(out=redsb[:], in_=red[:], func=AF.Copy,
                                             scale=0.01)
                        nc.gpsimd.indirect_dma_start(
                            out=agg_in[:, :],
                            out_offset=bass.IndirectOffsetOnAxis(
                                ap=dsti[:, tcol:tcol + 1], axis=0),
                            in_=redsb[:], in_offset=None)

                # ---- AllReduce ----
                nc.gpsimd.collective_compute(
                    "AllReduce", ALU.add,
                    replica_groups=[list(range(NC8))],
                    ins=[agg_in[:, :]], outs=[agg_out[:, :]])

                # ---- node phase ----
                for nt in range(NNT):
                    sl = slice(nt * 512, (nt + 1) * 512)
                    ams = ps.tile([128, 512], DT.float32, tag="scat")
                    amv = ps.tile([48, 512], DT.float32, tag="red")
                    for j in range(4):
                        r0 = nt * 512 + j * 128
                        ag = gat.tile([128, 176], DT.float32, tag="ag")
                        nc.sync.dma_start(out=ag[:], in_=agg_out[r0:r0 + 128, :])
                        MM(out=ams[:, j * 128:(j + 1) * 128], lhsT=ag[:, 0:128],
                           rhs=ident[:], is_transpose=True, start=True, stop=True)
                        MM(out=amv[:, j * 128:(j + 1) * 128], lhsT=ag[:, 128:176],
                           rhs=ident[:], is_transpose=True, start=True, stop=True)
                    s_sb = sb.tile([128, 512], DT.float32, tag="snode")
                    nc.sync.dma_start(out=s_sb[:], in_=TAB_S[:, sl])
                    pre = sb.tile([128, 512], DT.float32, tag="pre")
                    nc.vector.tensor_add(out=pre[:], in0=s_sb[:], in1=ams[:])
                    vf = sb.tile([48, 512], DT.float32, tag="vf")
                    nc.sync.dma_start(out=vf[:], in_=TAB_VF[:, sl])
                    vmid = sb.tile([48, 512], DT.float32, tag="vmid")
                    nc.vector.tensor_add(out=vmid[:], in0=vf[:], in1=amv[:])
                    s_ln = sb.tile([128, 512], DT.float32, tag="sln")
                    ln_cols(pre[:], f'c{ci}lnmg', f'c{ci}lnmb', s_ln[:])
                    # upd GVPs
                    us1 = sb.tile([128, 512], DT.float32, tag="us1")
                    sigu1, vuu1 = gvp(f'c{ci}u0', 16, 16,
                                      [(f'c{ci}u0Wls', s_ln[:])], vmid[:], None, us1[:])
                    uv1 = sb.tile([48, 512], DT.float32, tag="uv1")
                    nc.vector.tensor_mul(out=uv1[:], in0=sigu1[:], in1=vuu1[:])
                    us3 = sb.tile([128, 512], DT.float32, tag="us3")
                    uv3 = sb.tile([48, 512], DT.float32, tag="uv3")
                    gvp_chain2(f'c{ci}u1', f'c{ci}u2', us1[:], uv1[:], us3[:], uv3[:])
                    pre2 = sb.tile([128, 512], DT.float32, tag="pre2")
                    nc.vector.tensor_add(out=pre2[:], in0=s_ln[:], in1=us3[:])
                    s_out = sb.tile([128, 512], DT.float32, tag="sfin")
                    ln_cols(pre2[:], f'c{ci}lnug', f'c{ci}lnub', s_out[:])
                    v_out = sb.tile([48, 512], DT.float32, tag="vfin")
                    nc.vector.tensor_add(out=v_out[:], in0=vmid[:], in1=uv3[:])

                    if ci < 3:
                        nc.sync.dma_start(out=TAB_S[:, sl], in_=s_out[:])
                        nc.sync.dma_start(out=TAB_VF[:, sl], in_=v_out[:])
                        z1 = ps.tile([128, 512], DT.float32, tag="lin")
                        MM(out=z1[:], lhsT=W(f'c{ci + 1}z1w'), rhs=s_out[:],
                           start=True, stop=True)
                        z1s = sb.tile([128, 512], DT.float32, tag="z1s")
                        nc.vector.tensor_copy(out=z1s[:], in_=z1[:])
                        write_nodemaj(z1s[:], TAB_Z1, nt, 128)
                        write_nodemaj(v_out[:], TAB_V, nt, 48)

                    if ci in (1, 3):
                        # position update GVPs
                        ps1_ = sb.tile([128, 512], DT.float32, tag="ps1t")
                        sigp1, vup1 = gvp('p0', 16, 16, [('p0Wls', s_out[:])],
                                          v_out[:], None, ps1_[:])
                        pv1 = sb.tile([48, 512], DT.float32, tag="pv1")
                        nc.vector.tensor_mul(out=pv1[:], in0=sigp1[:], in1=vup1[:])
                        ps2_ = sb.tile([128, 512], DT.float32, tag="ps2t")
                        sigp2, vup2 = gvp('p1', 16, 16, [('p1Wls', ps1_[:])],
                                          pv1[:], None, ps2_[:])
                        pv2 = sb.tile([48, 512], DT.float32, tag="pv2")
                        nc.vector.tensor_mul(out=pv2[:], in0=sigp2[:], in1=vup2[:])
                        ps3_ = sb.tile([128, 512], DT.float32, tag="ps3t")
                        sigp3, vup3 = gvp('p2', 16, 1, [('p2Wls', ps2_[:])],
                                          pv2[:], None, ps3_[:], sigmoid_gate=False)
                        dx = sb.tile([3, 512], DT.float32, tag="dx")
                        nc.vector.tensor_mul(out=dx[:], in0=sigp3[:3, :], in1=vup3[:3, :])
                        xb = sb.tile([3, 512], DT.float32, tag="xb")
                        nc.sync.dma_start(out=xb[:], in_=XF[:, sl])
                        nc.vector.tensor_add(out=xb[:], in0=xb[:], in1=dx[:])
                        nc.sync.dma_start(out=XF[:, sl], in_=xb[:])
                        # za/zb tables for edge update
                        for wnm, tab in [('euA', TAB_ZA), ('euB', TAB_ZB)]:
                            zp = ps.tile([128, 512], DT.float32, tag="lin")
                            MM(out=zp[:], lhsT=W(wnm), rhs=s_out[:], start=True, stop=True)
                            zs = sb.tile([128, 512], DT.float32, tag="z1s")
                            nc.vector.tensor_copy(out=zs[:], in_=zp[:])
                            write_nodemaj(zs[:], tab, nt, 128)
                        if ci == 1:
                            # refresh TAB_X for dist recompute
                            for j in range(4):
                                r0 = nt * 512 + j * 128
                                xt = ps.tile([128, 128], DT.float32, tag="wtp")
                                MM(out=xt[:, :3], lhsT=xb[:, j * 128:(j + 1) * 128],
                                   rhs=ident[0:3, 0:3], is_transpose=True,
                                   start=True, stop=True)
                                xe = sb.tile([128, 128], DT.float32, tag="wev")
                                nc.vector.tensor_copy(out=xe[:, :3], in_=xt[:, :3])
                                nc.sync.dma_start(out=TAB_X[r0:r0 + 128, :], in_=xe[:, :3])

                    if ci == 3:
                        # node head
                        hp = ps.tile([128, 512], DT.float32, tag="lin")
                        MM(out=hp[:], lhsT=W('nh_l1W'), rhs=s_out[:], start=True, stop=True)
                        hh = sb.tile([128, 512], DT.float32, tag="h1")
                        nc.scalar.activation(out=hh[:], in_=hp[:], func=AF.Silu,
                                             bias=W('nh_l1b')[:, :])
                        op = ps.tile([16, 512], DT.float32, tag="wtp")
                        MM(out=op[:], lhsT=W('nh_l2W'), rhs=hh[:], start=True, stop=True)
                        ob = sb.tile([16, 512], DT.float32, tag="nhsb")
                        nc.scalar.activation(out=ob[:], in_=op[:], func=AF.Identity,
                                             bias=W('nh_l2b')[:, :])
                        for j in range(4):
                            r0 = nt * 512 + j * 128
                            tp = ps.tile([128, 128], DT.float32, tag="wtp")
                            MM(out=tp[:, :16], lhsT=ob[:, j * 128:(j + 1) * 128],
                               rhs=ident[:16, :16], is_transpose=True, start=True, stop=True)
                            ev = sb.tile([128, 128], DT.float32, tag="wev")
                            nc.vector.tensor_copy(out=ev[:, :16], in_=tp[:, :16])
                            nc.sync.dma_start(out=out_nh[r0:r0 + 128, :], in_=ev[:, :16])
                            # x output
                            xt = ps.tile([128, 128], DT.float32, tag="wtp")
                            MM(out=xt[:, :3], lhsT=xb[:, j * 128:(j + 1) * 128],
                               rhs=ident[0:3, 0:3], is_transpose=True,
                               start=True, stop=True)
                            xe = sb.tile([128, 128], DT.float32, tag="wev")
                            nc.vector.tensor_copy(out=xe[:, :3], in_=xt[:, :3])
                            nc.sync.dma_start(out=out_x[r0:r0 + 128, :], in_=xe[:, :3])

                # ---- edge update phase ----
                if ci in (1, 3):
                    for st in range(NST):
                        sl = slice(st * 512, (st + 1) * 512)
                        za_ts, zb_ts = [], []
                        for j in range(4):
                            tcol = st * 4 + j
                            za = gat.tile([128, 128], DT.float32, tag="zg")
                            nc.gpsimd.indirect_dma_start(
                                out=za[:], out_offset=None, in_=TAB_ZA[:, :],
                                in_offset=bass.IndirectOffsetOnAxis(
                                    ap=srci[:, tcol:tcol + 1], axis=0))
                            za_ts.append(za)
                            zb = gat.tile([128, 128], DT.float32, tag="zg2")
                            nc.gpsimd.indirect_dma_start(
                                out=zb[:], out_offset=None, in_=TAB_ZB[:, :],
                                in_offset=bass.IndirectOffsetOnAxis(
                                    ap=dsti[:, tcol:tcol + 1], axis=0))
                            zb_ts.append(zb)
                        ef_sb = sb2.tile([128, 512], DT.float32, tag="ef")
                        nc.sync.dma_start(out=ef_sb[:], in_=TAB_EF[:, sl])
                        lin = ps.tile([128, 512], DT.float32, tag="lin")
                        MM(out=lin[:], lhsT=W('euC'), rhs=ef_sb[:],
                           start=True, stop=False, skip_group_check=True)
                        for j in range(4):
                            MM(out=lin[:, j * 128:(j + 1) * 128], lhsT=za_ts[j][:],
                               rhs=ident[:], is_transpose=True, start=False, stop=False,
                               skip_group_check=True)
                            MM(out=lin[:, j * 128:(j + 1) * 128], lhsT=zb_ts[j][:],
                               rhs=ident[:], is_transpose=True, start=False,
                               stop=(j == 3), skip_group_check=True)
                        h1 = sb.tile([128, 512], DT.float32, tag="h1")
                        nc.scalar.activation(out=h1[:], in_=lin[:], func=AF.Silu,
                                             bias=W('eub1')[:, :])
                        p2 = ps.tile([128, 512], DT.float32, tag="lin")
                        MM(out=p2[:], lhsT=W('eu_l2W'), rhs=h1[:], start=True, stop=True)
                        h2 = sb.tile([128, 512], DT.float32, tag="h2")
                        nc.scalar.activation(out=h2[:], in_=p2[:], func=AF.Silu,
                                             bias=W('eu_l2b')[:, :])
                        pre = sb.tile([128, 512], DT.float32, tag="pre")
                        nc.vector.tensor_add(out=pre[:], in0=ef_sb[:], in1=h2[:])
                        ef_new = sb.tile([128, 512], DT.float32, tag="efout")
                        ln_cols(pre[:], 'eu_lng', 'eu_lnb', ef_new[:])
                        nc.sync.dma_start(out=TAB_EF[:, sl], in_=ef_new[:])

                        if ci == 1:
                            # recompute x_diff / d for these 4 tiles
                            for j in range(4):
                                tcol = st * 4 + j
                                esl = slice(tcol * 128, (tcol + 1) * 128)
                                xs = gat.tile([128, 3], DT.float32, tag="xs")
                                nc.gpsimd.indirect_dma_start(
                                    out=xs[:], out_offset=None, in_=TAB_X[:, :],
                                    in_offset=bass.IndirectOffsetOnAxis(
                                        ap=srci[:, tcol:tcol + 1], axis=0))
                                xd = gat.tile([128, 3], DT.float32, tag="xd")
                                nc.gpsimd.indirect_dma_start(
                                    out=xd[:], out_offset=None, in_=TAB_X[:, :],
                                    in_offset=bass.IndirectOffsetOnAxis(
                                        ap=dsti[:, tcol:tcol + 1], axis=0))
                                df = sb.tile([128, 3], DT.float32, tag="df")
                                nc.vector.tensor_sub(out=df[:], in0=xs[:], in1=xd[:])
                                sq2 = sb.tile([128, 3], DT.float32, tag="dsq")
                                nc.vector.tensor_mul(out=sq2[:], in0=df[:], in1=df[:])
                                ss = sb.tile([128, 1], DT.float32, tag="dss")
                                nc.vector.tensor_reduce(
                                    out=ss[:], in_=sq2[:], op=ALU.add,
                                    axis=mybir.AxisListType.X)
                                dij = sb.tile([128, 1], DT.float32, tag="dij")
                                nc.scalar.activation(out=dij[:], in_=ss[:], func=AF.Sqrt,
                                                     bias=W('eps8')[:, :])
                                nc.vector.tensor_scalar_add(out=dij[:], in0=dij[:],
                                                            scalar1=1e-8)
                                inv = sb.tile([128, 1], DT.float32, tag="inv")
                                nc.vector.reciprocal(out=inv[:], in_=dij[:])
                                xdf_et = sb.tile([128, 3], DT.float32, tag="xdfe")
                                nc.vector.tensor_mul(out=xdf_et[:], in0=df[:],
                                                     in1=inv[:].to_broadcast([128, 3]))
                                # rbf: exp(-((dij-mu)/sigma)^2)
                                dmu = sb.tile([128, 16], DT.float32, tag="dmu")
                                nc.vector.tensor_sub(
                                    out=dmu[:], in0=dij[:].to_broadcast([128, 16]),
                                    in1=W('mu16'))
                                sigma = RBF_DMAX / RBF
                                nc.scalar.activation(out=dmu[:], in_=dmu[:],
                                                     func=AF.Square, scale=1.0 / sigma)
                                d_et = sb.tile([128, 16], DT.float32, tag="det")
                                nc.scalar.activation(out=d_et[:], in_=dmu[:],
                                                     func=AF.Exp, scale=-1.0)
                                # transposes back into aux
                                tp = ps.tile([128, 128], DT.float32, tag="wtp")
                                MM(out=tp[:16, :], lhsT=d_et[:], rhs=ident[:],
                                   is_transpose=True, start=True, stop=True)
                                ev16 = sb.tile([16, 128], DT.float32, tag="ev16")
                                nc.vector.tensor_copy(out=ev16[:], in_=tp[:16, :])
                                nc.sync.dma_start(out=AUXD2[0:16, esl], in_=ev16[:])
                                tp2 = ps.tile([128, 128], DT.float32, tag="wtp")
                                MM(out=tp2[:3, :], lhsT=xdf_et[:], rhs=ident[:],
                                   is_transpose=True, start=True, stop=True)
                                ev3 = sb.tile([3, 128], DT.float32, tag="ev3")
                                nc.vector.tensor_copy(out=ev3[:], in_=tp2[:3, :])
                                nc.sync.dma_start(out=AUXD2[32:35, esl], in_=ev3[:])

            # ---------- edge head ----------
            for st in range(NST):
                sl = slice(st * 512, (st + 1) * 512)
                ef_sb = sb2.tile([128, 512], DT.float32, tag="ef")
                nc.sync.dma_start(out=ef_sb[:], in_=TAB_EF[:, sl])
                for j in range(4):
                    tp = ps.tile([128, 128], DT.float32, tag="wtp")
                    MM(out=tp[:], lhsT=ef_sb[:, j * 128:(j + 1) * 128], rhs=ident[:],
                       is_transpose=True, start=True, stop=True)
                    ev = sb.tile([128, 128], DT.float32, tag="wev")
                    nc.vector.tensor_copy(out=ev[:], in_=tp[:])
                    r0 = st * 512 + j * 128
                    nc.sync.dma_start(out=TAB_EFM[r0:r0 + 128, :], in_=ev[:])
            for ht in range(NROWS // 128):
                ue = gat.tile([128, 128], DT.float32, tag="ue")
                nc.gpsimd.indirect_dma_start(
                    out=ue[:], out_offset=None, in_=TAB_EFM[:, :],
                    in_offset=bass.IndirectOffsetOnAxis(ap=Ui[:, ht:ht + 1], axis=0))
                le = gat.tile([128, 128], DT.float32, tag="le")
                nc.gpsimd.indirect_dma_start(
                    out=le[:], out_offset=None, in_=TAB_EFM[:, :],
                    in_offset=bass.IndirectOffsetOnAxis(ap=Mi[:, ht:ht + 1], axis=0))
                ul = sb.tile([128, 128], DT.float32, tag="ul")
                nc.vector.tensor_add(out=ul[:], in0=ue[:], in1=le[:])
                tp = ps.tile([128, 128], DT.float32, tag="wtp")
                MM(out=tp[:], lhsT=ul[:], rhs=ident[:], is_transpose=True,
                   start=True, stop=True)
                ulT = sb.tile([128, 128], DT.float32, tag="ulT")
                nc.vector.tensor_copy(out=ulT[:], in_=tp[:])
                hp = ps.tile([128, 128], DT.float32, tag="lin")
                MM(out=hp[:], lhsT=W('eh_l1W'), rhs=ulT[:], start=True, stop=True)
                hh = sb.tile([128, 128], DT.float32, tag="ehh")
                nc.scalar.activation(out=hh[:], in_=hp[:], func=AF.Silu,
                                     bias=W('eh_l1b')[:, :])
                op = ps.tile([5, 128], DT.float32, tag="wtp")
                MM(out=op[:], lhsT=W('eh_l2W'), rhs=hh[:], start=True, stop=True)
                ob = sb.tile([5, 128], DT.float32, tag="ehob")
                nc.scalar.activation(out=ob[:], in_=op[:], func=AF.Identity,
                                     bias=W('eh_l2b')[:, :])
                tp2 = ps.tile([128, 128], DT.float32, tag="wtp")
                MM(out=tp2[:, :5], lhsT=ob[:], rhs=ident[:5, :5], is_transpose=True,
                   start=True, stop=True)
                ev = sb.tile([128, 128], DT.float32, tag="wev")
                nc.vector.tensor_copy(out=ev[:, :5], in_=tp2[:, :5])
                nc.sync.dma_start(out=out_el[ht * 128:(ht + 1) * 128, :], in_=ev[:, :5])

    nc.compile()
    return nc


# ================= entry point =================

def _get_runner(nc):
    if 'runner' in _CACHE:
        return _CACHE['runner']
    import jax
    from jax.sharding import Mesh, PartitionSpec
    from jax.experimental.shard_map import shard_map
    from concourse import bass2jax, mybir
    bass2jax.install_neuronx_cc_hook()
    partition_name = nc.partition_id_tensor.name if nc.partition_id_tensor else None
    in_names, out_names, out_avals, zero_outs = [], [], [], []
    for alloc in nc.m.functions[0].allocations:
        if not isinstance(alloc, mybir.MemoryLocationSet):
            continue
        name = alloc.memorylocations[0].name
        if alloc.kind == "ExternalInput":
            if name != partition_name:
                in_names.append(name)
        elif alloc.kind == "ExternalOutput":
            out_names.append(name)
            shape = tuple(alloc.tensor_shape)
            dtype = mybir.dt.np(alloc.dtype)
            out_avals.append(jax.core.ShapedArray(shape, dtype))
            zero_outs.append(np.zeros(shape, dtype))
    n_params = len(in_names)
    n_outs = len(out_avals)
    all_in_names = list(in_names) + list(out_names)
    if partition_name is not None:
        all_in_names.append(partition_name)
    donate = tuple(range(n_params, n_params + n_outs))

    def _body(*args):
        operands = list(args)
        if partition_name is not None:
            operands.append(bass2jax.partition_id_tensor())
        outs = bass2jax._bass_exec_p.bind(
            *operands,
            out_avals=tuple(out_avals),
            in_names=tuple(all_in_names),
            out_names=tuple(out_names),
            lowering_input_output_aliases=(),
            sim_require_finite=True,
            sim_require_nnan=True,
            nc=nc,
        )
        return tuple(outs)

    devices = jax.devices()[:NC8]
    mesh = Mesh(np.asarray(devices), ("core",))
    in_specs = (PartitionSpec("core"),) * (n_params + n_outs)
    out_specs = (PartitionSpec("core"),) * n_outs
    fn = jax.jit(
        shard_map(_body, mesh=mesh, in_specs=in_specs, out_specs=out_specs,
                  check_rep=False),
        donate_argnums=donate, keep_unused=True)
    runner = dict(fn=fn, in_names=in_names, out_names=out_names,
                  out_avals=out_avals, zero_outs=zero_outs, mesh=mesh,
                  n_params=n_params)
    _CACHE['runner'] = runner
    return runner


def _run(nc, in_maps):
    r = _get_runner(nc)
    concat_in = [
        np.concatenate([np.asarray(in_maps[c][name]) for c in range(NC8)], axis=0)
        for name in r['in_names']]
    concat_zeros = [np.zeros((NC8 * z.shape[0], *z.shape[1:]), z.dtype)
                    for z in r['zero_outs']]
    _CACHE['concat_in'] = concat_in
    out_arrs = r['fn'](*concat_in, *concat_zeros)
    results = [
        {name: np.asarray(out_arrs[i]).reshape(NC8, *r['out_avals'][i].shape)[c]
         for i, name in enumerate(r['out_names'])}
        for c in range(NC8)]
    return results


def bench(iters=5):
    """Time pure device execution with inputs pre-staged on device."""
    import jax, time
    from jax.sharding import NamedSharding, PartitionSpec
    r = _CACHE['runner']
    sh = NamedSharding(r['mesh'], PartitionSpec("core"))
    dev_in = [jax.device_put(x, sh) for x in _CACHE['concat_in']]
    zmake = lambda: [jax.device_put(
        np.zeros((NC8 * z.shape[0], *z.shape[1:]), z.dtype), sh)
        for z in r['zero_outs']]
    # warmup
    out = r['fn'](*dev_in, *zmake())
    jax.block_until_ready(out)
    ts = []
    for _ in range(iters):
        zs = zmake()
        jax.block_until_ready(zs)
        t0 = time.perf_counter()
        out = r['fn'](*dev_in, *zs)
        jax.block_until_ready(out)
        ts.append(time.perf_counter() - t0)
    return min(ts), ts


def kernel(**inputs):
    pk = pack_weights(inputs['params'])
    B = build_blob(pk)
    wb_arr = B.finalize()
    cores, s0T, x0T = prep(inputs)

    key = ('prog', wb_arr.shape[1])
    if key not in _CACHE:
        _CACHE[key] = build_program(B.entries, wb_arr.shape[1])
    nc = _CACHE[key]

    in_maps = []
    for c in range(NC8):
        co = cores[c]
        in_maps.append({
            'WB': wb_arr, 's0T': s0T, 'x0T': x0T, 'aux0': co['aux0'],
            'dstf': co['dstf'],
            'srci': co['srci'], 'dsti': co['dsti'], 'Ui': co['Ui'], 'Mi': co['Mi'],
            'e0T': co['e0T'],
        })
    results = _run(nc, in_maps)
    _CACHE['last_results'] = results

    r0 = results[0]
    x = r0['out_x'][:N].astype(F32)
    a_logits = r0['out_nh'][:N, :10].astype(F32)
    c_logits = r0['out_nh'][:N, 10:16].astype(F32)
    e_logits = np.concatenate(
        [results[c]['out_el'][:EC // 2] for c in range(NC8)], 0).astype(F32)
    return x, a_logits, c_logits, e_logits


# revision 43
# speedup vs baseline: 69.0680x; 1.5333x over previous
"""Trainium2 Bass kernel for nn_EndpointVectorField (GVP message-passing GNN).

Strategy (8 NeuronCores, SPMD):
  - Edges sharded by pair: core c owns upper edges [c*10k,(c+1)*10k) and their
    reverse mates, sorted by dst and packed into 128-edge tiles that never split
    a dst group (enables race-free scatter via selection-matrix matmul).
  - All activations feature-on-partition [128, cols]; node tables in DRAM are
    gathered per edge via indirect DMA and transposed on the PE.
  - Per-node aggregation: per-core partial tables scattered to DRAM, AllReduce
    across cores, node update replicated on every core.
  - Vector channel (GVP) path packed as block-diagonal matmuls over the 3
    coords with Wh and Wh@Wu fused side by side.
"""
import sys
import numpy as np

if '/opt/trn_rl_repo' not in sys.path:
    sys.path.insert(0, '/opt/trn_rl_repo')

# ---- problem constants (hardcoded per contract) ----
N = 10000
EH = 80000
E = 2 * EH
HS = 128
RBF = 16
RBF_DMAX = 20.0
NC8 = 8
EC = E // NC8          # 20000 edges/core
NT = 164               # 128-edge tiles per core
EP = NT * 128          # 20992 padded edges
NST = NT // 4          # 41 super-tiles of 512
NROWS = 10240          # padded node rows
NNT = NROWS // 512     # 20 node super-tiles
TRASH = N
LN_EPS = 1e-5
F32 = np.float32

_CACHE = {}


def f32(x):
    return np.asarray(x, dtype=F32)


# ================= weight packing (validated in np sim) =================

def pack_gvp(p, vi_has_extra, dh, vo):
    Wh = f32(p['Wh']); Wu = f32(p['Wu'])
    WhWu = f32(Wh @ Wu)
    v_off = 1 if vi_has_extra else 0
    Wvh = np.zeros((48, 3 * dh), F32)
    Wvu = np.zeros((48, 3 * vo), F32)
    for c in range(3):
        for v in range(16):
            Wvh[c * 16 + v, c * dh:(c + 1) * dh] = Wh[v_off + v]
            Wvu[c * 16 + v, c * vo:(c + 1) * vo] = WhWu[v_off + v]
    Wvhu = np.zeros((48, 64 + 3 * vo), F32)
    Wvhu[:, :3 * dh] = Wvh
    Wvhu[:, 64:] = Wvu
    out = {'Wvhu': Wvhu}
    if vi_has_extra:
        Wvh_x = np.zeros((3, 3 * dh), F32)
        Wvu_x = np.zeros((3, 3 * vo), F32)
        for c in range(3):
            Wvh_x[c, c * dh:(c + 1) * dh] = Wh[0]
            Wvu_x[c, c * vo:(c + 1) * vo] = WhWu[0]
        Wvhu_x = np.zeros((3, 64 + 3 * vo), F32)
        Wvhu_x[:, :3 * dh] = Wvh_x
        Wvhu_x[:, 64:] = Wvu_x
        out['Wvhu_x'] = Wvhu_x
    W = f32(p['lin']['W'])
    fi = W.shape[0] - dh
    out['Wlin_s'] = W[:fi]
    out['Wlin_sh'] = W[fi:]
    out['blin'] = f32(p['lin']['b'])
    Wg = f32(p['gate']['W']); bg = f32(p['gate']['b'])
    Wg_rep = np.zeros((Wg.shape[0], 3 * vo), F32)
    bg_rep = np.zeros((3 * vo,), F32)
    for c in range(3):
        Wg_rep[:, c * vo:(c + 1) * vo] = Wg
        bg_rep[c * vo:(c + 1) * vo] = bg
    out['Wg'] = Wg_rep
    out['bg'] = bg_rep
    out['dh'] = dh; out['vo'] = vo
    return out


def pack_weights(params):
    pk = {}
    for nm, src in [('scal_l1', params['scal_emb']['l1']),
                    ('scal_l2', params['scal_emb']['l2']),
                    ('edge_l1', params['edge_emb']['l1']),
                    ('edge_l2', params['edge_emb']['l2']),
                    ('eu_l2', params['edge_upd']['l2']),
                    ('nh_l1', params['node_head']['l1']),
                    ('nh_l2', params['node_head']['l2']),
                    ('eh_l1', params['edge_head']['l1']),
                    ('eh_l2', params['edge_head']['l2'])]:
        pk[nm] = (f32(src['W']), f32(src['b']))
    for nm, src in [('scal_ln', params['scal_emb']['ln']),
                    ('edge_ln', params['edge_emb']['ln']),
                    ('eu_ln', params['edge_upd']['ln'])]:
        pk[nm] = (f32(src['g']), f32(src['b']))
    pk['convs'] = []
    for ci in range(4):
        cv = params['convs'][ci]
        msg1 = pack_gvp(cv['msg'][0], True, 17, 16)
        W = msg1['Wlin_s']
        msg1['Ws'] = W[:HS].copy()
        msg1['Wd'] = W[HS:HS + RBF].copy()
        msg1['Wef'] = W[HS + RBF:].copy()
        msg = [msg1, pack_gvp(cv['msg'][1], False, 16, 16),
               pack_gvp(cv['msg'][2], False, 16, 16)]
        upd = [pack_gvp(cv['upd'][k], False, 16, 16) for k in range(3)]
        pk['convs'].append({
            'msg': msg, 'upd': upd,
            'ln_msg': (f32(cv['ln_msg']['g']), f32(cv['ln_msg']['b'])),
            'ln_upd': (f32(cv['ln_upd']['g']), f32(cv['ln_upd']['b']))})
    pk['pos'] = [pack_gvp(params['pos_upd'][0], False, 16, 16),
                 pack_gvp(params['pos_upd'][1], False, 16, 16),
                 pack_gvp(params['pos_upd'][2], False, 16, 1)]
    eu = params['edge_upd']
    W1 = f32(eu['l1']['W'])
    pk['eu_A'] = W1[:HS].copy()
    pk['eu_B'] = W1[HS:2 * HS].copy()
    pk['eu_C'] = W1[2 * HS:].copy()
    pk['eu_b1'] = f32(eu['l1']['b'])
    return pk


class Blob:
    """Packs 2-D f32 matrices into one [128, cols] SBUF-resident blob."""

    def __init__(self):
        self.cols = 0
        self.entries = {}   # name -> (row0, K, col0, M)
        self.arrays = {}

    def add(self, name, arr, row0=0):
        arr = f32(arr)
        if arr.ndim == 1:
            arr = arr[:, None]
        K, M = arr.shape
        assert row0 + K <= 128
        self.entries[name] = (row0, K, self.cols, M)
        self.arrays[name] = arr
        self.cols += M
        return name

    def finalize(self):
        buf = np.zeros((128, self.cols), F32)
        for name, (r0, K, c0, M) in self.entries.items():
            buf[r0:r0 + K, c0:c0 + M] = self.arrays[name]
        return buf


def build_blob(pk):
    B = Blob()
    B.add('ones128', np.ones((128, 1), F32))
    B.add('ones1', np.ones((1, 128), F32))
    ssel17 = np.zeros((51, 17), F32)
    for c in range(3):
        ssel17[c * 17:(c + 1) * 17] = np.eye(17, dtype=F32)
    B.add('ssel17', ssel17)
    ssel16 = np.zeros((48, 16), F32)
    for c in range(3):
        ssel16[c * 16:(c + 1) * 16] = np.eye(16, dtype=F32)
    B.add('ssel16', ssel16)
    mu = np.linspace(0.0, RBF_DMAX, RBF, dtype=F32)
    B.add('mu16', np.broadcast_to(mu[None, :], (128, RBF)).copy())
    B.add('eps8', np.full((128, 1), 1e-8, F32))
    B.add('epsln', np.full((128, 1), LN_EPS, F32))

    def add_gvp(pref, g):
        B.add(pref + 'Wvhu', g['Wvhu'])
        if 'Wvhu_x' in g:
            B.add(pref + 'Wvhu_x', g['Wvhu_x'], row0=32)
        if 'Ws' not in g and g['Wlin_s'].shape[0] in (128, 16):
            B.add(pref + 'Wls', g['Wlin_s'])
        B.add(pref + 'Wlsh', g['Wlin_sh'])
        B.add(pref + 'blin', g['blin'])
        B.add(pref + 'Wg', g['Wg'])
        B.add(pref + 'bg', g['bg'])

    for ci in range(4):
        cv = pk['convs'][ci]
        m1 = cv['msg'][0]
        B.add(f'c{ci}z1w', m1['Ws'])
        B.add(f'c{ci}m1Wd', m1['Wd'])
        B.add(f'c{ci}m1Wef', m1['Wef'])
        add_gvp(f'c{ci}m1', m1)
        add_gvp(f'c{ci}m2', cv['msg'][1])
        add_gvp(f'c{ci}m3', cv['msg'][2])
        for k in range(3):
            add_gvp(f'c{ci}u{k}', cv['upd'][k])
        B.add(f'c{ci}lnmg', cv['ln_msg'][0])
        B.add(f'c{ci}lnmb', cv['ln_msg'][1])
        B.add(f'c{ci}lnug', cv['ln_upd'][0])
        B.add(f'c{ci}lnub', cv['ln_upd'][1])
    for k in range(3):
        add_gvp(f'p{k}', pk['pos'][k])
    B.add('euA', pk['eu_A'])
    B.add('euB', pk['eu_B'])
    B.add('euC', pk['eu_C'])
    B.add('eub1', pk['eu_b1'])
    for nm in ['eu_l2', 'nh_l1', 'nh_l2', 'eh_l1', 'eh_l2',
               'scal_l1', 'scal_l2', 'edge_l1', 'edge_l2']:
        B.add(nm + 'W', pk[nm][0])
        B.add(nm + 'b', pk[nm][1])
    for nm in ['scal_ln', 'edge_ln', 'eu_ln']:
        B.add(nm + 'g', pk[nm][0])
        B.add(nm + 'b', pk[nm][1])
    return B


# ================= host prep =================

def prep(inputs):
    src = np.asarray(inputs['src_idx']).astype(np.int64)
    dst = np.asarray(inputs['dst_idx']).astype(np.int64)
    e_t = f32(inputs['e_t'])
    a_t = f32(inputs['a_t']); c_t = f32(inputs['c_t'])
    x_t = f32(inputs['x_t']); t = f32(inputs['t'])
    nbi = np.asarray(inputs['node_batch_idx']).astype(np.int64)

    s0 = np.zeros((N, 17), F32)
    s0[:, :10] = a_t
    s0[:, 10] = t[nbi]
    s0[:, 11:] = c_t
    s0T = np.zeros((17, NROWS), F32)
    s0T[:, :N] = s0.T
    x0T = np.zeros((3, NROWS), F32)
    x0T[:, :N] = x_t.T

    mu = np.linspace(0.0, RBF_DMAX, RBF, dtype=F32)
    sigma = F32(RBF_DMAX / RBF)

    cores = []
    for c in range(NC8):
        PH = EC // 2
        gidx = np.concatenate([np.arange(c * PH, (c + 1) * PH),
                               EH + np.arange(c * PH, (c + 1) * PH)])
        sc = src[gidx]; dc = dst[gidx]
        order = np.argsort(dc, kind='stable')
        ds = dc[order]
        groups = []
        run = 0
        for i in range(1, len(ds) + 1):
            if i == len(ds) or ds[i] != ds[i - 1]:
                groups.append((run, i)); run = i
        tiles = []
        cur = []; cur_len = 0
        for (a, b) in groups:
            gl = b - a
            assert gl <= 128, f"in-degree {gl} > 128"
            if cur_len + gl > 128:
                tiles.append(np.concatenate(cur)); cur = []; cur_len = 0
            cur.append(order[a:b]); cur_len += gl
        if cur_len:
            tiles.append(np.concatenate(cur))
        assert len(tiles) <= NT, f"core {c}: {len(tiles)} tiles > {NT}"
        src_p = np.zeros(EP, np.int32)
        dst_p = np.full(EP, TRASH, np.int32)
        e0_p = np.zeros((EP, 5), F32)
        gid_p = np.full(EP, -1, np.int64)
        for ti, tl in enumerate(tiles):
            n = len(tl)
            src_p[ti * 128:ti * 128 + n] = sc[tl]
            dst_p[ti * 128:ti * 128 + n] = dc[tl]
            e0_p[ti * 128:ti * 128 + n] = e_t[gidx[tl]]
            gid_p[ti * 128:ti * 128 + n] = gidx[tl]
        pos_of = {}
        for p_, g_ in enumerate(gid_p):
            if g_ >= 0:
                pos_of[g_] = p_
        up = np.arange(c * PH, (c + 1) * PH)
        U = np.zeros(NROWS, np.int32)
        M = np.zeros(NROWS, np.int32)
        U[:PH] = [pos_of[g_] for g_ in up]
        M[:PH] = [pos_of[g_ + EH] for g_ in up]

        diff = (x_t[src_p] - np.where((dst_p < N)[:, None],
                                      x_t[np.minimum(dst_p, N - 1)], 0.0)).astype(F32)
        dij = (np.sqrt((diff * diff).sum(1) + F32(1e-8)) + F32(1e-8)).astype(F32)
        xdf = (diff / dij[:, None]).astype(F32)
        d0 = np.exp(-(((dij[:, None] - mu[None, :]) / sigma) ** 2)).astype(F32)

        aux0 = np.zeros((35, EP), F32)
        aux0[0:16] = d0.T
        aux0[32:35] = xdf.T

        cores.append({
            'aux0': aux0,
            'dstf': dst_p.astype(F32).reshape(NT, 128).T.copy(),
            'srci': src_p.reshape(NT, 128).T.copy(),
            'dsti': dst_p.reshape(NT, 128).T.copy(),
            'Ui': U.reshape(NROWS // 128, 128).T.copy(),
            'Mi': M.reshape(NROWS // 128, 128).T.copy(),
            'e0T': np.ascontiguousarray(e0_p.T),
        })
    return cores, s0T, x0T


# ================= device program =================

def build_program(blob_entries, wcols, with_collectives=True):
    import concourse.bass as bass
    import concourse.bacc as bacc
    import concourse.tile as tile
    from concourse import mybir
    from concourse.masks import make_identity

    AF = mybir.ActivationFunctionType
    ALU = mybir.AluOpType
    DT = mybir.dt

    nc = bacc.Bacc("TRN2", target_bir_lowering=False, debug=False,
                   num_devices=NC8, enable_asserts=False)

    # ---- I/O ----
    WB = nc.dram_tensor("WB", [128, wcols], DT.float32, kind="ExternalInput")
    s0T = nc.dram_tensor("s0T", [17, NROWS], DT.float32, kind="ExternalInput")
    aux0 = nc.dram_tensor("aux0", [35, EP], DT.float32, kind="ExternalInput")
    x0T_in = nc.dram_tensor("x0T", [3, NROWS], DT.float32, kind="ExternalInput")
    dstf_in = nc.dram_tensor("dstf", [128, NT], DT.float32, kind="ExternalInput")
    srci_in = nc.dram_tensor("srci", [128, NT], DT.int32, kind="ExternalInput")
    dsti_in = nc.dram_tensor("dsti", [128, NT], DT.int32, kind="ExternalInput")
    Ui_in = nc.dram_tensor("Ui", [128, NROWS // 128], DT.int32, kind="ExternalInput")
    Mi_in = nc.dram_tensor("Mi", [128, NROWS // 128], DT.int32, kind="ExternalInput")
    e0T_in = nc.dram_tensor("e0T", [5, EP], DT.float32, kind="ExternalInput")
    ZER_in = nc.dram_tensor("ZER", [NROWS, 176], DT.float32, kind="ExternalInput")

    out_x = nc.dram_tensor("out_x", [NROWS, 3], DT.float32, kind="ExternalOutput")
    out_nh = nc.dram_tensor("out_nh", [NROWS, 16], DT.float32, kind="ExternalOutput")
    out_el = nc.dram_tensor("out_el", [NROWS, 5], DT.float32, kind="ExternalOutput")
    dbg_z1 = nc.dram_tensor("dbg_z1", [128, NROWS], DT.float32, kind="ExternalOutput")
    dbg_ef0 = nc.dram_tensor("dbg_ef0", [128, EP], DT.float32, kind="ExternalOutput")
    dbg_agg = nc.dram_tensor("dbg_agg", [NROWS, 176], DT.float32, kind="ExternalOutput")
    dbg_s1 = nc.dram_tensor("dbg_s1", [128, NROWS], DT.float32, kind="ExternalOutput")
    dbg_eff = nc.dram_tensor("dbg_eff", [128, EP], DT.float32, kind="ExternalOutput")

    # ---- internal DRAM tables ----
    TAB_NODE = nc.dram_tensor("TAB_NODE", [NROWS, 176], DT.float32, kind="Internal")
    TAB_ZA = nc.dram_tensor("TAB_ZA", [NROWS, 132], DT.float32, kind="Internal")
    TAB_ZB = nc.dram_tensor("TAB_ZB", [NROWS, 132], DT.float32, kind="Internal")
    TAB_S = nc.dram_tensor("TAB_S", [128, NROWS], DT.float32, kind="Internal")
    TAB_VF = nc.dram_tensor("TAB_VF", [48, NROWS], DT.float32, kind="Internal")
    TAB_EF = nc.dram_tensor("TAB_EF", [128, EP], DT.float32, kind="Internal")
    TAB_EFM = nc.dram_tensor("TAB_EFM", [EP, 128], DT.float32, kind="Internal")
    AUXD2 = nc.dram_tensor("AUXD2", [35, EP], DT.float32, kind="Internal")
    XF = nc.dram_tensor("XF", [3, NROWS], DT.float32, kind="Internal")
    AGG_IN = [nc.dram_tensor(f"AGG_IN{ci}", [NROWS, 176], DT.float32, kind="Internal")
              for ci in range(4)]
    AGG_OUT = [nc.dram_tensor(f"AGG_OUT{ci}", [NROWS, 176], DT.float32,
                              kind="Internal", addr_space="Shared")
               for ci in range(4)]

    with tile.TileContext(nc) as tc:
        from contextlib import ExitStack
        ctx = ExitStack()
        with ctx:
            persist = ctx.enter_context(tc.tile_pool(name="persist", bufs=1))
            sb = ctx.enter_context(tc.tile_pool(name="sb", bufs=1))
            sb2 = ctx.enter_context(tc.tile_pool(name="sb2", bufs=2))
            gat = ctx.enter_context(tc.tile_pool(name="gat", bufs=8))
            ps = ctx.enter_context(tc.tile_pool(name="ps", bufs=1, space="PSUM"))
            ps2 = ctx.enter_context(tc.tile_pool(name="ps2", bufs=2, space="PSUM"))

            wb = persist.tile([128, wcols], DT.float32)
            nc.sync.dma_start(out=wb[:], in_=WB[:, :])

            def W(name):
                r0, K, c0, M = blob_entries[name]
                return wb[r0:r0 + K, c0:c0 + M]

            dstf = persist.tile([128, NT], DT.float32)
            srci = persist.tile([128, NT], DT.int32)
            dsti = persist.tile([128, NT], DT.int32)
            Ui = persist.tile([128, NROWS // 128], DT.int32)
            Mi = persist.tile([128, NROWS // 128], DT.int32)
            for t_, i_ in [(dstf, dstf_in), (srci, srci_in), (dsti, dsti_in),
                           (Ui, Ui_in), (Mi, Mi_in)]:
                nc.sync.dma_start(out=t_[:], in_=i_[:, :])

            ident = persist.tile([128, 128], DT.float32)
            make_identity(nc, ident[:])
            zero_sb = persist.tile([128, 176], DT.float32)
            nc.vector.memset(zero_sb[:], 0.0)

            MM = nc.tensor.matmul

            # ---------- helpers ----------
            def ln_cols(pre_sb, gname, bname, out_t, n=512):
                """LayerNorm over partitions for [128, n] tile -> out_t."""
                stp = ps.tile([33, 512], DT.float32, tag="stats")
                stats = stp[0:1, :]
                sqs = stp[32:33, :]
                sq_sb = sb.tile([128, 512], DT.float32, tag="lnw")
                MM(out=stats[:, :n], lhsT=W('ones128'), rhs=pre_sb, start=True, stop=True)
                nc.scalar.activation(out=sq_sb[:, :n], in_=pre_sb, func=AF.Square)
                MM(out=sqs[:, :n], lhsT=W('ones128'), rhs=sq_sb[:, :n], start=True, stop=True)
                st_ = sb.tile([1, 2048], DT.float32, tag="lnst")
                m_sb = st_[0:1, 0:n]
                v_sb = st_[0:1, 512:512 + n]
                m2 = st_[0:1, 1024:1024 + n]
                r_sb = st_[0:1, 1536:1536 + n]
                nc.scalar.activation(out=m_sb, in_=stats[:, :n], func=AF.Copy,
                                     scale=1.0 / 128.0)
                nc.scalar.activation(out=v_sb, in_=sqs[:, :n], func=AF.Copy,
                                     scale=1.0 / 128.0)
                nc.vector.tensor_mul(out=m2, in0=m_sb, in1=m_sb)
                nc.vector.tensor_sub(out=v_sb, in0=v_sb, in1=m2)
                nc.scalar.activation(out=v_sb, in_=v_sb, func=AF.Sqrt,
                                     bias=W('epsln')[0:1, :])
                nc.vector.reciprocal(out=r_sb, in_=v_sb)
                mb = ps.tile([128, 512], DT.float32, tag="scat")
                rb = ps.tile([128, 512], DT.float32, tag="red")
                MM(out=mb[:, :n], lhsT=W('ones1'), rhs=m_sb, start=True, stop=True)
                MM(out=rb[:, :n], lhsT=W('ones1'), rhs=r_sb, start=True, stop=True)
                cs = sb.tile([128, 512], DT.float32, tag="lnw")
                nc.vector.tensor_sub(out=cs[:, :n], in0=pre_sb, in1=mb[:, :n])
                nc.vector.tensor_mul(out=cs[:, :n], in0=cs[:, :n], in1=rb[:, :n])
                nc.scalar.activation(out=out_t, in_=cs[:, :n], func=AF.Identity,
                                     scale=W(gname)[:, :], bias=W(bname)[:, :])

            def gvp(pref, dh, vo, lin_ins, mv_sb_ap, xdf_ap, out_ms, sigmoid_gate=True):
                """One GVP. lin_ins: list of (lhsT_name_or_ap, rhs_ap, K) matmul
                contributions plus optional ('T', src_tile) transpose contribs.
                mv_sb_ap: [48, 512] SBUF. Returns (gate_or_sig_sb, vu_psum)."""
                vhu = ps.tile([112, 512], DT.float32, tag="vh")
                wid = 64 + 3 * vo
                MM(out=vhu[:wid, :], lhsT=W(pref + 'Wvhu'), rhs=mv_sb_ap,
                   start=True, stop=(xdf_ap is None))
                if xdf_ap is not None:
                    MM(out=vhu[:wid, :], lhsT=W(pref + 'Wvhu_x'), rhs=xdf_ap,
                       start=False, stop=True)
                vh = vhu
                vu = vhu[64:wid, :]
                sq = sb2.tile([51, 512], DT.float32, tag="sq")
                nc.scalar.activation(out=sq[:3 * dh, :], in_=vhu[:3 * dh, :],
                                     func=AF.Square)
                ssq = ps.tile([17, 512], DT.float32, tag="gvaux")
                sselw = 'ssel17' if dh == 17 else 'ssel16'
                MM(out=ssq[:dh, :], lhsT=W(sselw), rhs=sq[:3 * dh, :], start=True, stop=True)
                sh = sb2.tile([17, 512], DT.float32, tag="sh")
                nc.scalar.activation(out=sh[:dh, :], in_=ssq[:dh, :], func=AF.Sqrt,
                                     bias=W('eps8')[0:dh, :])
                lin = ps2.tile([128, 512], DT.float32, tag="lin")
                first = True
                for item in lin_ins:
                    if item[0] == 'T':
                        assert not first, "transposes must accumulate after a start"
                        for j, zt in enumerate(item[1]):
                            MM(out=lin[:, j * 128:(j + 1) * 128], lhsT=zt,
                               rhs=ident[:], is_transpose=True,
                               start=False, stop=False, skip_group_check=True)
                    else:
                        lname, rhs_ap = item
                        MM(out=lin[:], lhsT=W(lname), rhs=rhs_ap,
                           start=first, stop=False, skip_group_check=True)
                        first = False
                MM(out=lin[:], lhsT=W(pref + 'Wlsh'), rhs=sh[:dh, :],
                   start=False, stop=True, skip_group_check=True)
                nc.scalar.activation(out=out_ms, in_=lin[:], func=AF.Silu,
                                     bias=W(pref + 'blin')[:, :])
                gate = ps.tile([48, 512], DT.float32, tag="gvaux")
                MM(out=gate[:3 * vo, :], lhsT=W(pref + 'Wg'), rhs=out_ms,
                   start=True, stop=True)
                sig = sb2.tile([48, 512], DT.float32, tag="sig")
                nc.scalar.activation(out=sig[:3 * vo, :], in_=gate[:3 * vo, :],
                                     func=AF.Sigmoid if sigmoid_gate else AF.Identity,
                                     bias=W(pref + 'bg')[:3 * vo, :])
                return sig, vu

            def gvp_chain2(pref2, pref3, ms_in, mv_in, out_ms, out_mv):
                """GVP2 then GVP3 (dh=vo=16), edge or node side."""
                ms2 = sb2.tile([128, 512], DT.float32, tag="ms2")
                sig2, vu2 = gvp(pref2, 16, 16, [(pref2 + 'Wls', ms_in)], mv_in, None, ms2[:])
                mv2 = sb2.tile([48, 512], DT.float32, tag="mv2")
                nc.vector.tensor_mul(out=mv2[:], in0=sig2[:], in1=vu2[:, :])
                sig3, vu3 = gvp(pref3, 16, 16, [(pref3 + 'Wls', ms2[:])], mv2[:], None, out_ms)
                nc.vector.tensor_mul(out=out_mv, in0=sig3[:], in1=vu3[:])

            def write_node2(a_sb, awidth, b_sb, bwidth, tab, rowlen, nt):
                """Two feature-major tiles ([awidth,512] + [bwidth,512]) ->
                node-major rows of tab [NROWS, rowlen] via one batched DMA."""
                ev4 = sb.tile([128, 4 * 176], DT.float32, tag="wev4")
                for j in range(4):
                    tp = ps.tile([128, 176], DT.float32, tag="wtp")
                    MM(out=tp[:, :awidth], lhsT=a_sb[:, j * 128:(j + 1) * 128],
                       rhs=ident[:awidth, :awidth], is_transpose=True,
                       start=True, stop=True)
                    if b_sb is not None:
                        MM(out=tp[:, awidth:awidth + bwidth],
                           lhsT=b_sb[:, j * 128:(j + 1) * 128],
                           rhs=ident[:bwidth, :bwidth], is_transpose=True,
                           start=True, stop=True)
                    nc.vector.tensor_copy(
                        out=ev4[:, j * rowlen:j * rowlen + rowlen],
                        in_=tp[:, :rowlen])
                out_view = tab[nt * 512:(nt + 1) * 512, :].rearrange(
                    "(j p) f -> p j f", j=4)
                in_view = ev4[:, :4 * rowlen].rearrange(
                    "p (j f) -> p j f", j=4)
                nc.sync.dma_start(out=out_view, in_=in_view)

            # ---------- init node phase: scal_emb ----------
            for nt in range(NNT):
                sl = slice(nt * 512, (nt + 1) * 512)
                s0_sb = sb.tile([17, 512], DT.float32, tag="s0")
                nc.sync.dma_start(out=s0_sb[:], in_=s0T[:, sl])
                p1 = ps2.tile([128, 512], DT.float32, tag="lin")
                MM(out=p1[:], lhsT=W('scal_l1W'), rhs=s0_sb[:], start=True, stop=True)
                h1 = sb.tile([128, 512], DT.float32, tag="h1")
                nc.scalar.activation(out=h1[:], in_=p1[:], func=AF.Silu,
                                     bias=W('scal_l1b')[:, :])
                p2 = ps2.tile([128, 512], DT.float32, tag="lin")
                MM(out=p2[:], lhsT=W('scal_l2W'), rhs=h1[:], start=True, stop=True)
                h2 = sb.tile([128, 512], DT.float32, tag="h2")
                nc.scalar.activation(out=h2[:], in_=p2[:], func=AF.Silu,
                                     bias=W('scal_l2b')[:, :])
                s_sb = sb.tile([128, 512], DT.float32, tag="sout")
                ln_cols(h2[:], 'scal_lng', 'scal_lnb', s_sb[:])
                nc.sync.dma_start(out=TAB_S[:, sl], in_=s_sb[:])
                z1 = ps2.tile([128, 512], DT.float32, tag="lin")
                MM(out=z1[:], lhsT=W('c0z1w'), rhs=s_sb[:], start=True, stop=True)
                z1s = sb.tile([128, 512], DT.float32, tag="z1s")
                nc.vector.tensor_copy(out=z1s[:], in_=z1[:])
                nc.sync.dma_start(out=dbg_z1[:, sl], in_=z1s[:])
                xb = sb.tile([3, 512], DT.float32, tag="xb")
                nc.sync.dma_start(out=xb[:], in_=x0T_in[:, sl])
                nc.sync.dma_start(out=XF[:, sl], in_=xb[:])
                vz = sb2.tile([48, 512], DT.float32, tag="mv3")
                nc.vector.memset(vz[:], 0.0)
                nc.sync.dma_start(out=TAB_VF[:, sl], in_=vz[:])
                write_node2(z1s[:], 128, vz[:], 48, TAB_NODE, 176, nt)

            # ---------- init edge phase: edge_emb ----------
            for st in range(NST):
                sl = slice(st * 512, (st + 1) * 512)
                e0_sb = sb.tile([5, 512], DT.float32, tag="e0")
                nc.sync.dma_start(out=e0_sb[:], in_=e0T_in[:, sl])
                p1 = ps2.tile([128, 512], DT.float32, tag="lin")
                MM(out=p1[:], lhsT=W('edge_l1W'), rhs=e0_sb[:], start=True, stop=True)
                h1 = sb.tile([128, 512], DT.float32, tag="h1")
                nc.scalar.activation(out=h1[:], in_=p1[:], func=AF.Silu,
                                     bias=W('edge_l1b')[:, :])
                p2 = ps2.tile([128, 512], DT.float32, tag="lin")
                MM(out=p2[:], lhsT=W('edge_l2W'), rhs=h1[:], start=True, stop=True)
                h2 = sb.tile([128, 512], DT.float32, tag="h2")
                nc.scalar.activation(out=h2[:], in_=p2[:], func=AF.Silu,
                                     bias=W('edge_l2b')[:, :])
                ef_sb = sb.tile([128, 512], DT.float32, tag="efout")
                ln_cols(h2[:], 'edge_lng', 'edge_lnb', ef_sb[:])
                nc.sync.dma_start(out=TAB_EF[:, sl], in_=ef_sb[:])
                nc.sync.dma_start(out=dbg_ef0[:, sl], in_=ef_sb[:])

            # ---------- conv loop ----------
            for ci in range(4):
                agg_in = AGG_IN[ci]
                agg_out = AGG_OUT[ci]
                # zero the partial table (DRAM->DRAM from zeros input)
                nc.sync.dma_start(out=agg_in[:, :], in_=ZER_in[:, :])

                # ---- edge message phase ----
                for st in range(NST):
                    sl = slice(st * 512, (st + 1) * 512)
                    zv_ts = []
                    for j in range(4):
                        tcol = st * 4 + j
                        zv = gat.tile([128, 176], DT.float32, tag="zg")
                        nc.gpsimd.indirect_dma_start(
                            out=zv[:], out_offset=None, in_=TAB_NODE[:, :],
                            in_offset=bass.IndirectOffsetOnAxis(
                                ap=srci[:, tcol:tcol + 1], axis=0))
                        zv_ts.append(zv)
                    ef_sb = sb2.tile([128, 512], DT.float32, tag="ef")
                    nc.scalar.dma_start(out=ef_sb[:], in_=TAB_EF[:, sl])
                    dxf = sb2.tile([35, 512], DT.float32, tag="dxf")
                    nc.scalar.dma_start(out=dxf[:],
                                        in_=(aux0 if ci < 2 else AUXD2)[:, sl])
                    # V transpose -> mv0 [48, 512]
                    vtp = ps.tile([48, 512], DT.float32, tag="gvaux")
                    for j in range(4):
                        MM(out=vtp[:, j * 128:(j + 1) * 128],
                           lhsT=zv_ts[j][:, 128:176],
                           rhs=ident[:], is_transpose=True, start=True, stop=True)
                    mv0 = sb2.tile([48, 512], DT.float32, tag="mv0")
                    nc.vector.tensor_copy(out=mv0[:], in_=vtp[:])

                    ms1 = sb2.tile([128, 512], DT.float32, tag="ms1")
                    sig1, vu1 = gvp(
                        f'c{ci}m1', 17, 16,
                        [(f'c{ci}m1Wef', ef_sb[:]),
                         (f'c{ci}m1Wd', dxf[0:16, :]),
                         ('T', [z[:, 0:128] for z in zv_ts])],
                        mv0[:], dxf[32:35, :], ms1[:])
                    mv1 = sb2.tile([48, 512], DT.float32, tag="mv1")
                    nc.vector.tensor_mul(out=mv1[:], in0=sig1[:], in1=vu1[:])
                    ms3 = sb2.tile([128, 512], DT.float32, tag="ms3")
                    mv3 = sb2.tile([48, 512], DT.float32, tag="mv3")
                    gvp_chain2(f'c{ci}m2', f'c{ci}m3', ms1[:], mv1[:], ms3[:], mv3[:])

                    # ---- scatter ----
                    for j in range(4):
                        tcol = st * 4 + j
                        sp = ps.tile([128, 512], DT.float32, tag="scat")
                        # dstT
                        MM(out=sp[:, 0:128],
                           lhsT=dstf[:, tcol:tcol + 1].to_broadcast([128, 128]),
                           rhs=ident[:], is_transpose=True, start=True, stop=True)
                        dstT = sb.tile([128, 128], DT.float32, tag="dstT")
                        nc.vector.tensor_copy(out=dstT[:], in_=sp[:, 0:128])
                        sel = sb.tile([128, 128], DT.float32, tag="sel")
                        nc.vector.tensor_tensor(
                            out=sel[:],
                            in0=dstf[:, tcol:tcol + 1].to_broadcast([128, 128]),
                            in1=dstT[:], op=ALU.is_equal)
                        # ms/mv transposes
                        MM(out=sp[:, 128:256], lhsT=ms3[:, j * 128:(j + 1) * 128],
                           rhs=ident[:], is_transpose=True, start=True, stop=True)
                        MM(out=sp[:, 256:304], lhsT=mv3[:, j * 128:(j + 1) * 128],
                           rhs=ident[:48, :48], is_transpose=True, start=True, stop=True)
                        ets = sb.tile([128, 176], DT.float32, tag="ets")
                        nc.vector.tensor_copy(out=ets[:], in_=sp[:, 128:304])
                        red = ps.tile([128, 176], DT.float32, tag="red")
                        MM(out=red[:, 0:176], lhsT=sel[:], rhs=ets[:, 0:176],
                           start=True, stop=True)
                        redsb = sb.tile([128, 176], DT.float32, tag="redsb")
                        nc.scalar.activation(out=redsb[:], in_=red[:], func=AF.Copy,
                                             scale=0.01)
                        nc.gpsimd.indirect_dma_start(
                            out=agg_in[:, :],
                            out_offset=bass.IndirectOffsetOnAxis(
                                ap=dsti[:, tcol:tcol + 1], axis=0),
                            in_=redsb[:], in_offset=None)

                # ---- AllReduce ----
                if with_collectives:
                    nc.gpsimd.collective_compute(
                        "AllReduce", ALU.add,
                        replica_groups=[list(range(NC8))],
                        ins=[agg_in[:, :]], outs=[agg_out[:, :]])
                else:
                    for r in range(NROWS // 128):
                        nc.sync.dma_start(
                            out=agg_out[r * 128:(r + 1) * 128, :],
                            in_=agg_in[r * 128:(r + 1) * 128, :])

                # ---- node phase ----
                for nt in range(NNT):
                    sl = slice(nt * 512, (nt + 1) * 512)
                    ams = ps.tile([128, 512], DT.float32, tag="scat")
                    amv = ps.tile([48, 512], DT.float32, tag="red")
                    for j in range(4):
                        r0 = nt * 512 + j * 128
                        ag = gat.tile([128, 176], DT.float32, tag="ag")
                        nc.scalar.dma_start(out=ag[:], in_=agg_out[r0:r0 + 128, :])
                        if ci == 0:
                            nc.sync.dma_start(out=dbg_agg[r0:r0 + 128, :], in_=ag[:])
                        MM(out=ams[:, j * 128:(j + 1) * 128], lhsT=ag[:, 0:128],
                           rhs=ident[:], is_transpose=True, start=True, stop=True)
                        MM(out=amv[:, j * 128:(j + 1) * 128], lhsT=ag[:, 128:176],
                           rhs=ident[:], is_transpose=True, start=True, stop=True)
                    s_sb = sb.tile([128, 512], DT.float32, tag="snode")
                    nc.scalar.dma_start(out=s_sb[:], in_=TAB_S[:, sl])
                    pre = sb.tile([128, 512], DT.float32, tag="pre")
                    nc.vector.tensor_add(out=pre[:], in0=s_sb[:], in1=ams[:])
                    vf = sb.tile([48, 512], DT.float32, tag="vf")
                    nc.scalar.dma_start(out=vf[:], in_=TAB_VF[:, sl])
                    vmid = sb.tile([48, 512], DT.float32, tag="vmid")
                    nc.vector.tensor_add(out=vmid[:], in0=vf[:], in1=amv[:])
                    s_ln = sb.tile([128, 512], DT.float32, tag="sln")
                    ln_cols(pre[:], f'c{ci}lnmg', f'c{ci}lnmb', s_ln[:])
                    # upd GVPs
                    us1 = sb.tile([128, 512], DT.float32, tag="us1")
                    sigu1, vuu1 = gvp(f'c{ci}u0', 16, 16,
                                      [(f'c{ci}u0Wls', s_ln[:])], vmid[:], None, us1[:])
                    uv1 = sb.tile([48, 512], DT.float32, tag="uv1")
                    nc.vector.tensor_mul(out=uv1[:], in0=sigu1[:], in1=vuu1[:])
                    us3 = sb.tile([128, 512], DT.float32, tag="us3")
                    uv3 = sb.tile([48, 512], DT.float32, tag="uv3")
                    gvp_chain2(f'c{ci}u1', f'c{ci}u2', us1[:], uv1[:], us3[:], uv3[:])
                    pre2 = sb.tile([128, 512], DT.float32, tag="pre2")
                    nc.vector.tensor_add(out=pre2[:], in0=s_ln[:], in1=us3[:])
                    s_out = sb.tile([128, 512], DT.float32, tag="sfin")
                    ln_cols(pre2[:], f'c{ci}lnug', f'c{ci}lnub', s_out[:])
                    v_out = sb.tile([48, 512], DT.float32, tag="vfin")
                    nc.vector.tensor_add(out=v_out[:], in0=vmid[:], in1=uv3[:])

                    if ci == 0:
                        nc.sync.dma_start(out=dbg_s1[:, sl], in_=s_out[:])
                    if ci < 3:
                        nc.sync.dma_start(out=TAB_S[:, sl], in_=s_out[:])
                        nc.sync.dma_start(out=TAB_VF[:, sl], in_=v_out[:])
                        z1 = ps2.tile([128, 512], DT.float32, tag="lin")
                        MM(out=z1[:], lhsT=W(f'c{ci + 1}z1w'), rhs=s_out[:],
                           start=True, stop=True)
                        z1s = sb.tile([128, 512], DT.float32, tag="z1s")
                        nc.vector.tensor_copy(out=z1s[:], in_=z1[:])
                        write_node2(z1s[:], 128, v_out[:], 48, TAB_NODE, 176, nt)

                    if ci in (1, 3):
                        # position update GVPs
                        ps1_ = sb.tile([128, 512], DT.float32, tag="ps1t")
                        sigp1, vup1 = gvp('p0', 16, 16, [('p0Wls', s_out[:])],
                                          v_out[:], None, ps1_[:])
                        pv1 = sb.tile([48, 512], DT.float32, tag="pv1")
                        nc.vector.tensor_mul(out=pv1[:], in0=sigp1[:], in1=vup1[:])
                        ps2_ = sb.tile([128, 512], DT.float32, tag="ps2t")
                        sigp2, vup2 = gvp('p1', 16, 16, [('p1Wls', ps1_[:])],
                                          pv1[:], None, ps2_[:])
                        pv2 = sb.tile([48, 512], DT.float32, tag="pv2")
                        nc.vector.tensor_mul(out=pv2[:], in0=sigp2[:], in1=vup2[:])
                        ps3_ = sb.tile([128, 512], DT.float32, tag="ps3t")
                        sigp3, vup3 = gvp('p2', 16, 1, [('p2Wls', ps2_[:])],
                                          pv2[:], None, ps3_[:], sigmoid_gate=False)
                        dx = sb.tile([3, 512], DT.float32, tag="dx")
                        nc.vector.tensor_mul(out=dx[:], in0=sigp3[:3, :], in1=vup3[:, :])
                        xb = sb.tile([3, 512], DT.float32, tag="xb")
                        nc.sync.dma_start(out=xb[:], in_=XF[:, sl])
                        nc.vector.tensor_add(out=xb[:], in0=xb[:], in1=dx[:])
                        nc.sync.dma_start(out=XF[:, sl], in_=xb[:])
                        # za/zb tables (with x in cols 128:131) for edge update
                        for wnm, tab in [('euA', TAB_ZA), ('euB', TAB_ZB)]:
                            zp = ps2.tile([128, 512], DT.float32, tag="lin")
                            MM(out=zp[:], lhsT=W(wnm), rhs=s_out[:], start=True, stop=True)
                            zs = sb.tile([128, 512], DT.float32, tag="z1s")
                            nc.vector.tensor_copy(out=zs[:], in_=zp[:])
                            write_node2(zs[:], 128, xb[:], 3, tab, 132, nt)

                    if ci == 3:
                        # node head
                        hp = ps2.tile([128, 512], DT.float32, tag="lin")
                        MM(out=hp[:], lhsT=W('nh_l1W'), rhs=s_out[:], start=True, stop=True)
                        hh = sb.tile([128, 512], DT.float32, tag="h1")
                        nc.scalar.activation(out=hh[:], in_=hp[:], func=AF.Silu,
                                             bias=W('nh_l1b')[:, :])
                        op = ps.tile([16, 512], DT.float32, tag="wtp")
                        MM(out=op[:], lhsT=W('nh_l2W'), rhs=hh[:], start=True, stop=True)
                        ob = sb.tile([16, 512], DT.float32, tag="nhsb")
                        nc.scalar.activation(out=ob[:], in_=op[:], func=AF.Identity,
                                             bias=W('nh_l2b')[:, :])
                        for j in range(4):
                            r0 = nt * 512 + j * 128
                            tp = ps.tile([128, 128], DT.float32, tag="wtp")
                            MM(out=tp[:, :16], lhsT=ob[:, j * 128:(j + 1) * 128],
                               rhs=ident[:16, :16], is_transpose=True, start=True, stop=True)
                            ev = sb.tile([128, 128], DT.float32, tag="wev")
                            nc.vector.tensor_copy(out=ev[:, :16], in_=tp[:, :16])
                            nc.sync.dma_start(out=out_nh[r0:r0 + 128, :], in_=ev[:, :16])
                            # x output
                            xt = ps.tile([128, 128], DT.float32, tag="wtp")
                            MM(out=xt[:, :3], lhsT=xb[:, j * 128:(j + 1) * 128],
                               rhs=ident[0:3, 0:3], is_transpose=True,
                               start=True, stop=True)
                            xe = sb.tile([128, 128], DT.float32, tag="wev")
                            nc.vector.tensor_copy(out=xe[:, :3], in_=xt[:, :3])
                            nc.sync.dma_start(out=out_x[r0:r0 + 128, :], in_=xe[:, :3])

                # ---- edge update phase ----
                if ci in (1, 3):
                    for st in range(NST):
                        sl = slice(st * 512, (st + 1) * 512)
                        za_ts, zb_ts = [], []
                        for j in range(4):
                            tcol = st * 4 + j
                            za = gat.tile([128, 132], DT.float32, tag="zg")
                            nc.gpsimd.indirect_dma_start(
                                out=za[:], out_offset=None, in_=TAB_ZA[:, :],
                                in_offset=bass.IndirectOffsetOnAxis(
                                    ap=srci[:, tcol:tcol + 1], axis=0))
                            za_ts.append(za)
                            zb = gat.tile([128, 132], DT.float32, tag="zg2")
                            nc.gpsimd.indirect_dma_start(
                                out=zb[:], out_offset=None, in_=TAB_ZB[:, :],
                                in_offset=bass.IndirectOffsetOnAxis(
                                    ap=dsti[:, tcol:tcol + 1], axis=0))
                            zb_ts.append(zb)
                        ef_sb = sb2.tile([128, 512], DT.float32, tag="ef")
                        nc.scalar.dma_start(out=ef_sb[:], in_=TAB_EF[:, sl])
                        lin = ps2.tile([128, 512], DT.float32, tag="lin")
                        MM(out=lin[:], lhsT=W('euC'), rhs=ef_sb[:],
                           start=True, stop=False, skip_group_check=True)
                        for j in range(4):
                            MM(out=lin[:, j * 128:(j + 1) * 128],
                               lhsT=za_ts[j][:, 0:128],
                               rhs=ident[:], is_transpose=True, start=False, stop=False,
                               skip_group_check=True)
                            MM(out=lin[:, j * 128:(j + 1) * 128],
                               lhsT=zb_ts[j][:, 0:128],
                               rhs=ident[:], is_transpose=True, start=False,
                               stop=(j == 3), skip_group_check=True)
                        h1 = sb.tile([128, 512], DT.float32, tag="h1")
                        nc.scalar.activation(out=h1[:], in_=lin[:], func=AF.Silu,
                                             bias=W('eub1')[:, :])
                        p2 = ps2.tile([128, 512], DT.float32, tag="lin")
                        MM(out=p2[:], lhsT=W('eu_l2W'), rhs=h1[:], start=True, stop=True)
                        h2 = sb.tile([128, 512], DT.float32, tag="h2")
                        nc.scalar.activation(out=h2[:], in_=p2[:], func=AF.Silu,
                                             bias=W('eu_l2b')[:, :])
                        pre = sb.tile([128, 512], DT.float32, tag="pre")
                        nc.vector.tensor_add(out=pre[:], in0=ef_sb[:], in1=h2[:])
                        ef_new = sb.tile([128, 512], DT.float32, tag="efout")
                        ln_cols(pre[:], 'eu_lng', 'eu_lnb', ef_new[:])
                        nc.sync.dma_start(out=TAB_EF[:, sl], in_=ef_new[:])

                        if ci == 1:
                            # recompute x_diff / d (x rides in za/zb cols 128:131)
                            for j in range(4):
                                tcol = st * 4 + j
                                esl = slice(tcol * 128, (tcol + 1) * 128)
                                df = sb.tile([128, 3], DT.float32, tag="df")
                                nc.vector.tensor_sub(out=df[:],
                                                     in0=za_ts[j][:, 128:131],
                                                     in1=zb_ts[j][:, 128:131])
                                sq2 = sb.tile([128, 3], DT.float32, tag="dsq")
                                nc.vector.tensor_mul(out=sq2[:], in0=df[:], in1=df[:])
                                ss = sb.tile([128, 1], DT.float32, tag="dss")
                                nc.vector.tensor_reduce(
                                    out=ss[:], in_=sq2[:], op=ALU.add,
                                    axis=mybir.AxisListType.X)
                                dij = sb.tile([128, 1], DT.float32, tag="dij")
                                nc.scalar.activation(out=dij[:], in_=ss[:], func=AF.Sqrt,
                                                     bias=W('eps8')[:, :])
                                nc.vector.tensor_scalar_add(out=dij[:], in0=dij[:],
                                                            scalar1=1e-8)
                                inv = sb.tile([128, 1], DT.float32, tag="inv")
                                nc.vector.reciprocal(out=inv[:], in_=dij[:])
                                xdf_et = sb.tile([128, 3], DT.float32, tag="xdfe")
                                nc.vector.tensor_mul(out=xdf_et[:], in0=df[:],
                                                     in1=inv[:].to_broadcast([128, 3]))
                                # rbf: exp(-((dij-mu)/sigma)^2)
                                dmu = sb.tile([128, 16], DT.float32, tag="dmu")
                                nc.vector.tensor_sub(
                                    out=dmu[:], in0=dij[:].to_broadcast([128, 16]),
                                    in1=W('mu16'))
                                sigma = RBF_DMAX / RBF
                                nc.scalar.activation(out=dmu[:], in_=dmu[:],
                                                     func=AF.Square, scale=1.0 / sigma)
                                d_et = sb.tile([128, 16], DT.float32, tag="det")
                                nc.scalar.activation(out=d_et[:], in_=dmu[:],
                                                     func=AF.Exp, scale=-1.0)
                                # transposes back into aux
                                tp = ps.tile([128, 128], DT.float32, tag="wtp")
                                MM(out=tp[:16, :], lhsT=d_et[:], rhs=ident[:],
                                   is_transpose=True, start=True, stop=True)
                                ev16 = sb.tile([16, 128], DT.float32, tag="ev16")
                                nc.vector.tensor_copy(out=ev16[:], in_=tp[:16, :])
                                nc.sync.dma_start(out=AUXD2[0:16, esl], in_=ev16[:])
                                tp2 = ps.tile([128, 128], DT.float32, tag="wtp")
                                MM(out=tp2[:3, :], lhsT=xdf_et[:], rhs=ident[:],
                                   is_transpose=True, start=True, stop=True)
                                ev3 = sb.tile([3, 128], DT.float32, tag="ev3")
                                nc.vector.tensor_copy(out=ev3[:], in_=tp2[:3, :])
                                nc.sync.dma_start(out=AUXD2[32:35, esl], in_=ev3[:])

            # ---------- edge head ----------
            for st in range(NST):
                sl = slice(st * 512, (st + 1) * 512)
                ef_sb = sb2.tile([128, 512], DT.float32, tag="ef")
                nc.sync.dma_start(out=ef_sb[:], in_=TAB_EF[:, sl])
                nc.sync.dma_start(out=dbg_eff[:, sl], in_=ef_sb[:])
                for j in range(4):
                    tp = ps.tile([128, 128], DT.float32, tag="wtp")
                    MM(out=tp[:], lhsT=ef_sb[:, j * 128:(j + 1) * 128], rhs=ident[:],
                       is_transpose=True, start=True, stop=True)
                    ev = sb.tile([128, 128], DT.float32, tag="wev")
                    nc.vector.tensor_copy(out=ev[:], in_=tp[:])
                    r0 = st * 512 + j * 128
                    nc.sync.dma_start(out=TAB_EFM[r0:r0 + 128, :], in_=ev[:])
            for ht in range(NROWS // 128):
                ue = gat.tile([128, 128], DT.float32, tag="ue")
                nc.gpsimd.indirect_dma_start(
                    out=ue[:], out_offset=None, in_=TAB_EFM[:, :],
                    in_offset=bass.IndirectOffsetOnAxis(ap=Ui[:, ht:ht + 1], axis=0))
                le = gat.tile([128, 128], DT.float32, tag="le")
                nc.gpsimd.indirect_dma_start(
                    out=le[:], out_offset=None, in_=TAB_EFM[:, :],
                    in_offset=bass.IndirectOffsetOnAxis(ap=Mi[:, ht:ht + 1], axis=0))
                ul = sb.tile([128, 128], DT.float32, tag="ul")
                nc.vector.tensor_add(out=ul[:], in0=ue[:], in1=le[:])
                tp = ps.tile([128, 128], DT.float32, tag="wtp")
                MM(out=tp[:], lhsT=ul[:], rhs=ident[:], is_transpose=True,
                   start=True, stop=True)
                ulT = sb.tile([128, 128], DT.float32, tag="ulT")
                nc.vector.tensor_copy(out=ulT[:], in_=tp[:])
                hp = ps2.tile([128, 128], DT.float32, tag="lin")
                MM(out=hp[:], lhsT=W('eh_l1W'), rhs=ulT[:], start=True, stop=True)
                hh = sb.tile([128, 128], DT.float32, tag="ehh")
                nc.scalar.activation(out=hh[:], in_=hp[:], func=AF.Silu,
                                     bias=W('eh_l1b')[:, :])
                op = ps.tile([5, 128], DT.float32, tag="wtp")
                MM(out=op[:], lhsT=W('eh_l2W'), rhs=hh[:], start=True, stop=True)
                ob = sb.tile([5, 128], DT.float32, tag="ehob")
                nc.scalar.activation(out=ob[:], in_=op[:], func=AF.Identity,
                                     bias=W('eh_l2b')[:, :])
                tp2 = ps.tile([128, 128], DT.float32, tag="wtp")
                MM(out=tp2[:, :5], lhsT=ob[:], rhs=ident[:5, :5], is_transpose=True,
                   start=True, stop=True)
                ev = sb.tile([128, 128], DT.float32, tag="wev")
                nc.vector.tensor_copy(out=ev[:, :5], in_=tp2[:, :5])
                nc.sync.dma_start(out=out_el[ht * 128:(ht + 1) * 128, :], in_=ev[:, :5])

    nc.compile()
    return nc


# ================= entry point =================

def _get_runner(nc):
    if 'runner' in _CACHE:
        return _CACHE['runner']
    import jax
    from jax.sharding import Mesh, PartitionSpec
    from jax.experimental.shard_map import shard_map
    from concourse import bass2jax, mybir
    bass2jax.install_neuronx_cc_hook()
    partition_name = nc.partition_id_tensor.name if nc.partition_id_tensor else None
    in_names, out_names, out_avals, zero_outs = [], [], [], []
    for alloc in nc.m.functions[0].allocations:
        if not isinstance(alloc, mybir.MemoryLocationSet):
            continue
        name = alloc.memorylocations[0].name
        if alloc.kind == "ExternalInput":
            if name != partition_name:
                in_names.append(name)
        elif alloc.kind == "ExternalOutput":
            out_names.append(name)
            shape = tuple(alloc.tensor_shape)
            dtype = mybir.dt.np(alloc.dtype)
            out_avals.append(jax.core.ShapedArray(shape, dtype))
            zero_outs.append(np.zeros(shape, dtype))
    n_params = len(in_names)
    n_outs = len(out_avals)
    all_in_names = list(in_names) + list(out_names)
    if partition_name is not None:
        all_in_names.append(partition_name)
    donate = tuple(range(n_params, n_params + n_outs))

    def _body(*args):
        operands = list(args)
        if partition_name is not None:
            operands.append(bass2jax.partition_id_tensor())
        outs = bass2jax._bass_exec_p.bind(
            *operands,
            out_avals=tuple(out_avals),
            in_names=tuple(all_in_names),
            out_names=tuple(out_names),
            lowering_input_output_aliases=(),
            sim_require_finite=True,
            sim_require_nnan=True,
            nc=nc,
        )
        return tuple(outs)

    devices = jax.devices()[:NC8]
    mesh = Mesh(np.asarray(devices), ("core",))
    in_specs = (PartitionSpec("core"),) * (n_params + n_outs)
    out_specs = (PartitionSpec("core"),) * n_outs
    fn = jax.jit(
        shard_map(_body, mesh=mesh, in_specs=in_specs, out_specs=out_specs,
                  check_rep=False),
        donate_argnums=donate, keep_unused=True)
    runner = dict(fn=fn, in_names=in_names, out_names=out_names,
                  out_avals=out_avals, zero_outs=zero_outs, mesh=mesh,
                  n_params=n_params)
    _CACHE['runner'] = runner
    return runner


def _run(nc, in_maps):
    r = _get_runner(nc)
    concat_in = [
        np.concatenate([np.asarray(in_maps[c][name]) for c in range(NC8)], axis=0)
        for name in r['in_names']]
    concat_zeros = [np.zeros((NC8 * z.shape[0], *z.shape[1:]), z.dtype)
                    for z in r['zero_outs']]
    _CACHE['concat_in'] = concat_in
    out_arrs = r['fn'](*concat_in, *concat_zeros)
    results = [
        {name: np.asarray(out_arrs[i]).reshape(NC8, *r['out_avals'][i].shape)[c]
         for i, name in enumerate(r['out_names'])}
        for c in range(NC8)]
    return results


def bench(iters=5):
    """Time pure device execution with inputs pre-staged on device."""
    import jax, time
    from jax.sharding import NamedSharding, PartitionSpec
    r = _CACHE['runner']
    sh = NamedSharding(r['mesh'], PartitionSpec("core"))
    dev_in = [jax.device_put(x, sh) for x in _CACHE['concat_in']]
    zmake = lambda: [jax.device_put(
        np.zeros((NC8 * z.shape[0], *z.shape[1:]), z.dtype), sh)
        for z in r['zero_outs']]
    # warmup
    out = r['fn'](*dev_in, *zmake())
    jax.block_until_ready(out)
    ts = []
    for _ in range(iters):
        zs = zmake()
        jax.block_until_ready(zs)
        t0 = time.perf_counter()
        out = r['fn'](*dev_in, *zs)
        jax.block_until_ready(out)
        ts.append(time.perf_counter() - t0)
    return min(ts), ts


def kernel(**inputs):
    pk = pack_weights(inputs['params'])
    B = build_blob(pk)
    wb_arr = B.finalize()
    cores, s0T, x0T = prep(inputs)

    key = ('prog', wb_arr.shape[1])
    if key not in _CACHE:
        _CACHE[key] = build_program(B.entries, wb_arr.shape[1])
    nc = _CACHE[key]

    in_maps = []
    for c in range(NC8):
        co = cores[c]
        in_maps.append({
            'WB': wb_arr, 's0T': s0T, 'x0T': x0T, 'aux0': co['aux0'],
            'dstf': co['dstf'],
            'srci': co['srci'], 'dsti': co['dsti'], 'Ui': co['Ui'], 'Mi': co['Mi'],
            'e0T': co['e0T'], 'ZER': _CACHE.setdefault(
                'zer', np.zeros((NROWS, 176), F32)),
        })
    results = _run(nc, in_maps)
    _CACHE['last_results'] = results

    r0 = results[0]
    x = r0['out_x'][:N].astype(F32)
    a_logits = r0['out_nh'][:N, :10].astype(F32)
    c_logits = r0['out_nh'][:N, 10:16].astype(F32)
    e_logits = np.concatenate(
        [results[c]['out_el'][:EC // 2] for c in range(NC8)], 0).astype(F32)
    return x, a_logits, c_logits, e_logits
